# revision 17
# baseline (speedup 1.0000x reference)
"""Trainium2 Bass kernel for sliding-window GQA attention block.

Module: q/k/v projections -> per-head RMSNorm(q,k) -> RoPE -> sliding-window
causal attention (window=1024, GQA 16 q heads / 4 kv heads) -> out projection.

Sharding (8 cores, tensor parallel over heads):
  core c owns q heads {2c, 2c+1} and kv head c//2.
  Each core computes attention for its 2 heads and a partial out-projection
  (contraction over its 256 head-features); the host sums the 8 partials.

Layout strategy on-chip (per core):
  - x is passed transposed (xt [D, S]) so QKV projection produces
    qT/kT [head_dim=128 partitions, S free] directly with wide N=512 matmuls.
  - v is produced transposed too, then PE-transposed into natural [S, 128]
    blocks (needed as the stationary operand of the PV matmul).
  - scores are computed transposed: sT [kv, q] = K_blk @ Q^T so that
    P^T (post-exp) is directly the moving operand for PV.  The softmax
    denominator uses an all-ones [128,128] stationary operand, which both
    sums over kv AND broadcasts the result across all 128 partitions, so
    normalization is a purely local reciprocal+multiply (no DRAM bounce).
  - RMS norm + RoPE run in the transposed layout: rotate-half is applied via
    a partition-rotated copy (DMA) plus host-precomputed cos/sin tables that
    also fold in the q/k norm weights; the 1/rms row is broadcast across
    partitions with gpsimd.partition_broadcast.
  - sliding-window structure: boundary / window-edge kv blocks only compute
    their valid q column range (column pruning), saving ~25% of attention
    matmul + exp work.
  - the partial out-projection is interleaved per-q-tile into the attention
    phase (attention h0/h1 of tile t, then out rows of tile t-1), and the
    attention inner loop is software-pipelined (den/PV of block i emitted
    after QK of block i+1) so the in-order PE queue never idles on the exp.

Precision: matmul-fed tensors are bf16 (set KERNEL_DTYPE=f32 for full fp32);
RMS/softmax-denominator chains are fp32; the partial output is written bf16.
"""

import os
import sys

for _p in ("/opt/trn_rl_repo", "/root/.axon_site/_ro/trn_rl_repo"):
    if _p not in sys.path:
        sys.path.insert(0, _p)

import numpy as np

N_HEADS = 16
N_KV_HEADS = 4
HEAD_DIM = 128
D_MODEL = 2048
WINDOW = 1024
THETA = 10000.0
EPS = 1e-6
S = 2048
B = 1
N_CORES = 8
KD = D_MODEL // 128          # 16 contraction tiles over d_model
NQT = S // 512               # 4 q tiles of 512
SCALE = HEAD_DIM ** -0.5

DTYPE_MODE = os.environ.get("KERNEL_DTYPE", "bf16")

# mask tile ids by delta0 = qstart - kvstart
_MASK_D0 = [0, -128, -256, -384, 640, 768, 896, 1024]
_MASK_IDX = {d0: i for i, d0 in enumerate(_MASK_D0)}


def _blocks(t):
    """kv blocks for q tile t: (b, q_off, q_wid, mask_idx), widest first.

    Column pruning: a boundary block (d0<=0) only reaches q >= -d0; a
    window-edge block (d0>=640) only reaches q < 1152-d0.  The first
    (full-width) block carries start=True and zeroes the whole psum range.
    """
    out = []
    for b in range(max(0, 4 * t - 8), 4 * t + 4):
        d0 = 512 * t - 128 * b
        if d0 <= 0:
            off, wid = -d0, 512 + d0
        elif d0 >= 640:
            off, wid = 0, 1152 - d0
        else:
            off, wid = 0, 512
        mi = _MASK_IDX.get(d0) if (d0 <= 0 or d0 > 512) else None
        out.append((b, off, wid, mi))
    out.sort(key=lambda x: -x[2])
    return out


def _build_program():
    import concourse.bass as bass
    import concourse.bacc as bacc
    import concourse.tile as tile
    from concourse import mybir
    from concourse.masks import make_identity

    f32 = mybir.dt.float32
    sd = mybir.dt.bfloat16 if DTYPE_MODE == "bf16" else f32
    AF = mybir.ActivationFunctionType

    nc = bacc.Bacc("TRN2", target_bir_lowering=False, debug=False)

    # host-pretiled: xt_t[k][p][s] = x[s, 128k+p]; wcat_t[p] holds the
    # [kt, m] weight tiles for partition p; similarly wot_t / masks.
    xt_d = nc.dram_tensor("xt", [KD, 128, S], sd, kind="ExternalInput").ap()
    wcat_d = nc.dram_tensor("wcat", [128, KD, 512], sd, kind="ExternalInput").ap()
    wot_d = nc.dram_tensor("wot", [128, 2, D_MODEL], sd, kind="ExternalInput").ap()
    cs2q_d = nc.dram_tensor("cs2q", [128, S], sd, kind="ExternalInput").ap()
    ss2q_d = nc.dram_tensor("ss2q", [128, S], sd, kind="ExternalInput").ap()
    cs2k_d = nc.dram_tensor("cs2k", [128, S], sd, kind="ExternalInput").ap()
    ss2k_d = nc.dram_tensor("ss2k", [128, S], sd, kind="ExternalInput").ap()
    masks_d = nc.dram_tensor("masks", [128, 8, 512], sd, kind="ExternalInput").ap()
    out_d = nc.dram_tensor("out", [S, D_MODEL], sd, kind="ExternalOutput").ap()

    _dbg = bool(os.environ.get("KERNEL_DEBUG"))
    if _dbg:
        dbg_q0 = nc.dram_tensor("dbg_q0", [128, S], sd, kind="ExternalOutput").ap()
        dbg_q1 = nc.dram_tensor("dbg_q1", [128, S], sd, kind="ExternalOutput").ap()
        dbg_k = nc.dram_tensor("dbg_k", [128, S], sd, kind="ExternalOutput").ap()
        dbg_v = nc.dram_tensor("dbg_v", [128, KD, HEAD_DIM], sd, kind="ExternalOutput").ap()
        dbg_o0 = nc.dram_tensor("dbg_o0", [128, S], sd, kind="ExternalOutput").ap()
        dbg_o1 = nc.dram_tensor("dbg_o1", [128, S], sd, kind="ExternalOutput").ap()

    with tile.TileContext(nc) as tc:
        with tc.tile_pool(name="persist", bufs=1) as persist:
            # q0, q1, k transposed [128 hd, S]; start as pre-rope, finalized in place
            qkv = [persist.tile([128, S], sd, tag=f"qkv{m}", name=f"qkv{m}") for m in range(3)]
            vnat = persist.tile([128, KD, HEAD_DIM], sd, tag="vnat")
            oT = [persist.tile([128, S], sd, tag=f"oT{h}", name=f"oT{h}") for h in range(2)]
            ones_mat = persist.tile([128, 128], sd, tag="ones_mat")
            nc.vector.memset(ones_mat, 1.0)
            ident = persist.tile([128, 128], sd, tag="ident")
            make_identity(nc, ident)
            mean_col = persist.tile([128, 1], sd, tag="mean_col")
            nc.vector.memset(mean_col, 1.0 / HEAD_DIM)
            eps1 = persist.tile([1, 1], f32, tag="eps1")
            nc.vector.memset(eps1, EPS)
            eps128 = persist.tile([128, 1], f32, tag="eps128")
            nc.vector.memset(eps128, HEAD_DIM * EPS)
            # per-kv-block softmax scale: sclK[p, b] = SCALE / rms(k tok 128b+p)
            sclK = persist.tile([128, KD], f32, tag="sclK")

            ab_pools = (
                tc.tile_pool(name="rotp", bufs=1),
                tc.tile_pool(name="aw", bufs=1),
                tc.tile_pool(name="ax", bufs=1),
                tc.tile_pool(name="avt", bufs=2),
                tc.tile_pool(name="btmp", bufs=2),
                tc.tile_pool(name="bsm", bufs=2),
                tc.tile_pool(name="apsum", bufs=3, space="PSUM"),
                tc.tile_pool(name="bps", bufs=1, space="PSUM"),
                tc.tile_pool(name="bpsT", bufs=1, space="PSUM"),
            )
            rotp, awp, axp, avt, btp, bsm, apsum, bps, bpsT = (p.__enter__() for p in ab_pools)
            rot = [rotp.tile([128, S], sd, tag=f"rot{m}", name=f"rot{m}") for m in range(3)]
            wcat_sb = awp.tile([128, KD, 512], sd)
            xt_sb = axp.tile([128, KD, S], sd)
            for k in range(KD):
                nc.sync.dma_start(out=wcat_sb[:, k, :], in_=wcat_d[:, k, :])
                nc.sync.dma_start(out=xt_sb[:, k, :], in_=xt_d[k, :, :])

            # warm the ACT function tables while projections run
            dummy = persist.tile([1, 1], f32, tag="dummy")
            nc.scalar.activation(dummy, eps1, AF.Square)
            nc.scalar.activation(dummy, eps1, AF.Sqrt)
            nc.scalar.activation(dummy, eps1, AF.Exp)

            # constants for phases B/C/D (after the critical-path loads)
            masks_sb = persist.tile([128, 8, 512], sd, tag="masks_sb")
            nc.sync.dma_start(out=masks_sb, in_=masks_d)
            wot_sb = persist.tile([128, 2, D_MODEL], sd, tag="wot_sb")
            nc.sync.dma_start(out=wot_sb, in_=wot_d)
            cs_sb = {}
            for nm, dd in (("cs2q", cs2q_d), ("ss2q", ss2q_d),
                           ("cs2k", cs2k_d), ("ss2k", ss2k_d)):
                t_ = persist.tile([128, S], sd, tag=nm, name=nm)
                nc.sync.dma_start(out=t_, in_=dd)
                cs_sb[nm] = t_

            def proj_pass(m):
                """project one 128-row slice of W_cat: 64 matmuls + eviction"""
                for half in range(2):
                    Y = apsum.tile([128, 1024], f32, tag="Y", name="Y")
                    for k in range(KD):
                        for n2 in range(2):
                            n = 2 * half + n2
                            nc.tensor.matmul(
                                Y[:, n2 * 512:(n2 + 1) * 512],
                                lhsT=wcat_sb[:, k, m * 128:(m + 1) * 128],
                                rhs=xt_sb[:, k, n * 512:(n + 1) * 512],
                                start=(k == 0),
                                stop=(k == KD - 1),
                            )
                    hs = slice(half * 1024, (half + 1) * 1024)
                    if m != 3:
                        if half == 0:
                            nc.scalar.copy(qkv[m][:, hs], Y)
                        else:
                            nc.vector.tensor_copy(qkv[m][:, hs], Y)
                    else:
                        # v: transpose 128x128 blocks into natural [S, hd]
                        vtmp = avt.tile([128, 1024], sd, tag="vtmp")
                        nc.scalar.copy(vtmp, Y)
                        tp = apsum.tile([128, 1024], sd, tag="Y", name="tpY")
                        for j in range(8):
                            nc.tensor.transpose(
                                tp[:, j * 128:(j + 1) * 128],
                                vtmp[:, j * 128:(j + 1) * 128], ident,
                            )
                        nc.vector.tensor_copy(vnat[:, half * 8:(half + 1) * 8, :], tp)
                if m != 3:
                    nc.sync.dma_start(out=rot[m][64:128, :], in_=qkv[m][0:64, :])
                    nc.sync.dma_start(out=rot[m][0:64, :], in_=qkv[m][64:128, :])

            def norm_rope(m):
                """rms norm + rope, in place on qkv[m].

                q (m<2): rn is needed per qT COLUMN -> row chain: [1,S] rms,
                reciprocal in a folded [128,16] view (InstReciprocal cost
                scales with FREE size: ~13us at 512+, 0.2us at 16), unfold,
                partition_broadcast, multiply into qT.

                k (m=2): rn lands per PARTITION in the score layout, so it is
                deferred to the exp: sclK[p,b] = 1/sqrt(sum_sq + 128*eps)
                = SCALE/rms, computed via 16 transposed n=1 matmuls. Kills
                k's DMA/broadcast chain entirely (rope commutes with the
                per-token scalar rn).
                """
                pre = qkv[m]
                rt = rot[m]
                csx = cs_sb["cs2q" if m < 2 else "cs2k"]
                ssx = cs_sb["ss2q" if m < 2 else "ss2k"]
                if m == 2:
                    psT = bpsT.tile([128, KD], f32, tag="psT")
                    for t in range(NQT):
                        ts_ = slice(t * 512, (t + 1) * 512)
                        sq = btp.tile([128, 512], sd, tag="sq")
                        nc.scalar.activation(sq, pre[:, ts_], AF.Square)
                        for j in range(4):
                            b = 4 * t + j
                            nc.tensor.matmul(
                                psT[:, b:b + 1],
                                lhsT=sq[:, j * 128:(j + 1) * 128],
                                rhs=ones_mat[:, 0:1],
                            )
                    nc.scalar.activation(sclK, psT, AF.Sqrt, bias=eps128)
                    nc.vector.reciprocal(sclK, sclK)
                else:
                    ss_sb = bsm.tile([1, S], f32, tag="ss")
                    for t in range(NQT):
                        ts_ = slice(t * 512, (t + 1) * 512)
                        sq = btp.tile([128, 512], sd, tag="sq")
                        nc.scalar.activation(sq, pre[:, ts_], AF.Square)
                        ps = bps.tile([1, 512], f32, tag="ssps")
                        nc.tensor.matmul(ps, lhsT=mean_col, rhs=sq)
                        nc.scalar.activation(ss_sb[0:1, ts_], ps, AF.Sqrt, bias=eps1)
                    ssT = bsm.tile([128, KD], f32, tag="ssT")
                    nc.sync.dma_start(out=ssT, in_=ss_sb)
                    nc.vector.reciprocal(ssT, ssT)
                    nc.sync.dma_start(out=ss_sb, in_=ssT)
                    rnb = btp.tile([128, S], f32, tag="rnb")
                    nc.gpsimd.partition_broadcast(rnb, ss_sb)
                # rope: final = (pre*cs + rot*ss) [* rn for q]
                t1 = btp.tile([128, S], sd, tag="t1")
                nc.vector.tensor_mul(t1, pre, csx)
                t2 = btp.tile([128, S], sd, tag="t2")
                nc.vector.tensor_mul(t2, rt, ssx)
                nc.vector.tensor_add(pre, t1, t2)
                if m != 2:
                    nc.vector.tensor_mul(pre, pre, rnb)

            # interleaved emission: each tensor completes while the next
            # projection keeps the tensor engine busy
            proj_pass(2)      # k
            proj_pass(3)      # v (transposes overlap k's norm chain)
            norm_rope(2)
            proj_pass(0)      # q0
            norm_rope(0)
            proj_pass(1)      # q1
            norm_rope(1)

            for p in reversed(ab_pools):
                p.__exit__(None, None, None)

            if _dbg:
                nc.sync.dma_start(out=dbg_q0, in_=qkv[0])
                nc.sync.dma_start(out=dbg_q1, in_=qkv[1])
                nc.sync.dma_start(out=dbg_k, in_=qkv[2])
                nc.sync.dma_start(out=dbg_v, in_=vnat)

            # -------- Phase C: windowed attention + fused out projection -----
            kT = qkv[2]
            with (
                tc.tile_pool(name="cpt", bufs=3) as cptp,
                tc.tile_pool(name="csm", bufs=2) as csmp,
                tc.tile_pool(name="dout", bufs=2) as doutp,
                tc.tile_pool(name="csc", bufs=2, space="PSUM") as cscp,
                tc.tile_pool(name="cacc", bufs=2, space="PSUM") as caccp,
                tc.tile_pool(name="cden", bufs=2, space="PSUM") as cdenp,
                tc.tile_pool(name="dps", bufs=2, space="PSUM") as dpsp,
            ):
                # software pipeline state: den/PV of block i are emitted after
                # QK of block i+1 (even across tile boundaries) so the
                # in-order PE queue never waits for the exp of block i.
                pending = []      # closures emitting den/PV of the previous block
                done_group = []   # (h, t, psO, psDb) awaiting normalize

                def flush():
                    for fn in pending:
                        fn()
                    pending.clear()

                def finalize():
                    for h, t, psO, psDb in done_group:
                        ts_ = slice(t * 512, (t + 1) * 512)
                        rbs = csmp.tile([128, 512], f32, tag="rbs")
                        # den is >= exp(diag) > 0 and O(1..1024): safe for the
                        # fast approx (18 bits, ~5x cheaper than reciprocal)
                        nc.vector.reciprocal_approx_fast(rbs, psDb)
                        nc.vector.tensor_mul(oT[h][:, ts_], psO, rbs)
                    done_group.clear()

                def attn_tile(h, t):
                    qT = qkv[h]
                    ts0 = t * 512
                    blocks = _blocks(t)
                    psO = caccp.tile([128, 512], f32, tag="psO")
                    psDb = cdenp.tile([128, 512], f32, tag="psDb")
                    for i, (b, off, wid, mi) in enumerate(blocks):
                        ps = cscp.tile([128, 512], f32, tag="sc")
                        nc.tensor.matmul(
                            ps[:, :wid],
                            lhsT=kT[:, b * 128:(b + 1) * 128],
                            rhs=qT[:, ts0 + off:ts0 + off + wid],
                        )
                        flush()
                        if i == 0:
                            finalize()
                        pt = cptp.tile([128, 512], sd, tag="pt")
                        nc.scalar.activation(pt[:, :wid], ps[:, :wid], AF.Exp,
                                             scale=sclK[:, b:b + 1])
                        if mi is not None:
                            nc.vector.tensor_mul(
                                pt[:, :wid], pt[:, :wid],
                                masks_sb[:, mi, off:off + wid],
                            )

                        first = (i == 0)
                        last = (i == len(blocks) - 1)

                        def emit_pv(b=b, off=off, wid=wid, pt=pt,
                                    psO=psO, psDb=psDb, first=first, last=last):
                            nc.tensor.matmul(
                                psDb[:, off:off + wid], lhsT=ones_mat,
                                rhs=pt[:, :wid],
                                start=first, stop=last, skip_group_check=True,
                            )
                            nc.tensor.matmul(
                                psO[:, off:off + wid], lhsT=vnat[:, b, :],
                                rhs=pt[:, :wid],
                                start=first, stop=last, skip_group_check=True,
                            )
                        pending.append(emit_pv)
                    done_group.append((h, t, psO, psDb))

                def outproj(t):
                    for tq in range(4 * t, 4 * t + 4):
                        ob = doutp.tile([128, D_MODEL], sd, tag="ob")
                        for dn in range(4):
                            po = dpsp.tile([128, 512], f32, tag="po")
                            nc.tensor.matmul(
                                po,
                                lhsT=oT[0][:, tq * 128:(tq + 1) * 128],
                                rhs=wot_sb[:, 0, dn * 512:(dn + 1) * 512],
                                start=True, stop=False,
                            )
                            nc.tensor.matmul(
                                po,
                                lhsT=oT[1][:, tq * 128:(tq + 1) * 128],
                                rhs=wot_sb[:, 1, dn * 512:(dn + 1) * 512],
                                start=False, stop=True,
                            )
                            ds_ = slice(dn * 512, (dn + 1) * 512)
                            # gpsimd cannot read PSUM; split across DVE/ACT
                            if dn % 2 == 0:
                                nc.vector.tensor_copy(ob[:, ds_], po)
                            else:
                                nc.scalar.copy(ob[:, ds_], po)
                        nc.sync.dma_start(out=out_d[tq * 128:(tq + 1) * 128, :], in_=ob)

                # attention tiles interleaved with out rows of finished tiles;
                # h0 tiles lead so q1's norm chain hides under them.
                # outproj(t) may only be emitted after a LATER attn_tile has
                # run finalize() for both (0,t) and (1,t) — emission order is
                # dependency order.
                attn_tile(0, 3)
                attn_tile(0, 0)
                attn_tile(1, 3)
                attn_tile(1, 0)   # finalize(1,3) runs at this tile's head
                outproj(3)
                attn_tile(0, 1)   # finalize(1,0)
                outproj(0)
                attn_tile(1, 1)
                attn_tile(0, 2)   # finalize(1,1)
                outproj(1)
                attn_tile(1, 2)
                flush()
                finalize()
                outproj(2)

                if _dbg:
                    nc.sync.dma_start(out=dbg_o0, in_=oT[0])
                    nc.sync.dma_start(out=dbg_o1, in_=oT[1])

    nc.compile()
    return nc


def _host_shards(x, wq, wk, wv, wo, q_norm_w, k_norm_w):
    if DTYPE_MODE == "bf16":
        import ml_dtypes
        sdt = ml_dtypes.bfloat16
    else:
        sdt = np.float32

    x2 = np.asarray(x, np.float32).reshape(S, D_MODEL)
    # [KD, 128, S]: xt[k, p, s] = x[s, 128k+p]
    xt = np.ascontiguousarray(x2.T.reshape(KD, 128, S)).astype(sdt)

    inv = 1.0 / (THETA ** (np.arange(0, HEAD_DIM, 2, dtype=np.float64) / HEAD_DIM))
    ang = np.arange(S, dtype=np.float64)[:, None] * inv[None, :]  # [S, 64]
    cos = np.cos(ang).T.astype(np.float32)  # [64, S]
    sin = np.sin(ang).T.astype(np.float32)

    def cs_ss(w):
        w = np.asarray(w, np.float32)
        wrot = np.concatenate([w[64:], w[:64]])
        cs2 = np.concatenate([cos, cos], 0) * w[:, None]
        ss2 = np.concatenate([-sin, sin], 0) * wrot[:, None]
        return np.ascontiguousarray(cs2).astype(sdt), np.ascontiguousarray(ss2).astype(sdt)

    cs2q, ss2q = cs_ss(q_norm_w)
    cs2k, ss2k = cs_ss(k_norm_w)

    masks = np.zeros((8, 128, 512), np.float32)
    ii = np.arange(128)[:, None]
    jj = np.arange(512)[None, :]
    for mi, d0 in enumerate(_MASK_D0):
        d = d0 + jj - ii
        masks[mi] = ((d >= 0) & (d < WINDOW)).astype(np.float32)
    masks_t = np.ascontiguousarray(masks.transpose(1, 0, 2)).astype(sdt)  # [128, 8, 512]

    wq = np.asarray(wq, np.float32)
    wk = np.asarray(wk, np.float32)
    wv = np.asarray(wv, np.float32)
    wo = np.asarray(wo, np.float32)

    in_maps = []
    for c in range(N_CORES):
        g = c // 2
        wcat = np.concatenate(
            [wq[256 * c:256 * (c + 1)], wk[128 * g:128 * (g + 1)], wv[128 * g:128 * (g + 1)]],
            axis=0,
        )  # [512, D]
        wcat_t = np.ascontiguousarray(
            wcat.T.reshape(KD, 128, 512).transpose(1, 0, 2)
        ).astype(sdt)  # [128, KD, 512]
        wot_t = np.ascontiguousarray(
            wo[:, 256 * c:256 * (c + 1)].T.reshape(2, 128, D_MODEL).transpose(1, 0, 2)
        ).astype(sdt)  # [128, 2, D]
        in_maps.append({
            "xt": xt,
            "wcat": wcat_t,
            "wot": wot_t,
            "cs2q": cs2q, "ss2q": ss2q, "cs2k": cs2k, "ss2k": ss2k,
            "masks": masks_t,
        })
    return in_maps


_NC_CACHE = None


def run_with_results(x, wq, wk, wv, wo, q_norm_w, k_norm_w, trace=False):
    global _NC_CACHE
    from concourse.bass_utils import run_bass_kernel_spmd

    if _NC_CACHE is None:
        _NC_CACHE = _build_program()
    nc = _NC_CACHE
    in_maps = _host_shards(x, wq, wk, wv, wo, q_norm_w, k_norm_w)
    res = run_bass_kernel_spmd(nc, in_maps, list(range(N_CORES)), trace=trace)
    parts = np.stack([np.asarray(res.results[i]["out"], np.float32) for i in range(N_CORES)], axis=0)
    out = parts.sum(axis=0, dtype=np.float32).reshape(B, S, D_MODEL)
    return np.ascontiguousarray(out.astype(np.float32)), res


def kernel(x, wq, wk, wv, wo, q_norm_w, k_norm_w):
    out, _ = run_with_results(x, wq, wk, wv, wo, q_norm_w, k_norm_w, trace=False)
    return out


# revision 21
# speedup vs baseline: 1.0106x; 1.0106x over previous
"""Trainium2 Bass kernel for sliding-window GQA attention block.

Module: q/k/v projections -> per-head RMSNorm(q,k) -> RoPE -> sliding-window
causal attention (window=1024, GQA 16 q heads / 4 kv heads) -> out projection.

Sharding (8 cores, tensor parallel over heads):
  core c owns q heads {2c, 2c+1} and kv head c//2.
  Each core computes attention for its 2 heads and a partial out-projection
  (contraction over its 256 head-features); the host sums the 8 partials.

Layout strategy on-chip (per core):
  - x is passed transposed (xt [D, S]) so QKV projection produces
    qT/kT [head_dim=128 partitions, S free] directly with wide N=512 matmuls.
  - v is produced transposed too, then PE-transposed into natural [S, 128]
    blocks (needed as the stationary operand of the PV matmul).
  - scores are computed transposed: sT [kv, q] = K_blk @ Q^T so that
    P^T (post-exp) is directly the moving operand for PV.  The softmax
    denominator uses an all-ones [128,128] stationary operand, which both
    sums over kv AND broadcasts the result across all 128 partitions, so
    normalization is a purely local reciprocal+multiply (no DRAM bounce).
  - RMS norm + RoPE run in the transposed layout: rotate-half is applied via
    a partition-rotated copy (DMA) plus host-precomputed cos/sin tables that
    also fold in the q/k norm weights; the 1/rms row is broadcast across
    partitions with gpsimd.partition_broadcast.
  - sliding-window structure: boundary / window-edge kv blocks only compute
    their valid q column range (column pruning), saving ~25% of attention
    matmul + exp work.
  - the partial out-projection is interleaved per-q-tile into the attention
    phase (attention h0/h1 of tile t, then out rows of tile t-1), and the
    attention inner loop is software-pipelined (den/PV of block i emitted
    after QK of block i+1) so the in-order PE queue never idles on the exp.

Precision: matmul-fed tensors are bf16 (set KERNEL_DTYPE=f32 for full fp32);
RMS/softmax-denominator chains are fp32; the partial output is written bf16.
"""

import os
import sys

for _p in ("/opt/trn_rl_repo", "/root/.axon_site/_ro/trn_rl_repo"):
    if _p not in sys.path:
        sys.path.insert(0, _p)

import numpy as np

N_HEADS = 16
N_KV_HEADS = 4
HEAD_DIM = 128
D_MODEL = 2048
WINDOW = 1024
THETA = 10000.0
EPS = 1e-6
S = 2048
B = 1
N_CORES = 8
KD = D_MODEL // 128          # 16 contraction tiles over d_model
NQT = S // 512               # 4 q tiles of 512
SCALE = HEAD_DIM ** -0.5

DTYPE_MODE = os.environ.get("KERNEL_DTYPE", "bf16")

# mask tile ids by delta0 = qstart - kvstart
_MASK_D0 = [0, -128, -256, -384, 640, 768, 896, 1024]
_MASK_IDX = {d0: i for i, d0 in enumerate(_MASK_D0)}


def _blocks(t):
    """kv blocks for q tile t: (b, q_off, q_wid, mask_idx), widest first.

    Column pruning: a boundary block (d0<=0) only reaches q >= -d0; a
    window-edge block (d0>=640) only reaches q < 1152-d0.  The first
    (full-width) block carries start=True and zeroes the whole psum range.
    """
    out = []
    for b in range(max(0, 4 * t - 8), 4 * t + 4):
        d0 = 512 * t - 128 * b
        if d0 <= 0:
            off, wid = -d0, 512 + d0
        elif d0 >= 640:
            off, wid = 0, 1152 - d0
        else:
            off, wid = 0, 512
        mi = _MASK_IDX.get(d0) if (d0 <= 0 or d0 > 512) else None
        out.append((b, off, wid, mi))
    out.sort(key=lambda x: -x[2])
    return out


def _build_program():
    import concourse.bass as bass
    import concourse.bacc as bacc
    import concourse.tile as tile
    from concourse import mybir
    from concourse.masks import make_identity

    f32 = mybir.dt.float32
    sd = mybir.dt.bfloat16 if DTYPE_MODE == "bf16" else f32
    AF = mybir.ActivationFunctionType

    nc = bacc.Bacc("TRN2", target_bir_lowering=False, debug=False)

    # host-pretiled: xt_t[p][k][s] = x[s, 128k+p] (partition-major so SBUF
    # loads are large contiguous 2D DMAs); wcat_t[p] holds the [kt, m]
    # weight tiles for partition p; similarly wot_t / masks.
    xt_d = nc.dram_tensor("xt", [128, KD, S], sd, kind="ExternalInput").ap()
    wcat_d = nc.dram_tensor("wcat", [128, KD, 512], sd, kind="ExternalInput").ap()
    wot_d = nc.dram_tensor("wot", [128, 2, D_MODEL], sd, kind="ExternalInput").ap()
    cs2q_d = nc.dram_tensor("cs2q", [128, S], sd, kind="ExternalInput").ap()
    ss2q_d = nc.dram_tensor("ss2q", [128, S], sd, kind="ExternalInput").ap()
    cs2k_d = nc.dram_tensor("cs2k", [128, S], sd, kind="ExternalInput").ap()
    ss2k_d = nc.dram_tensor("ss2k", [128, S], sd, kind="ExternalInput").ap()
    masks_d = nc.dram_tensor("masks", [128, 8, 512], sd, kind="ExternalInput").ap()
    out_d = nc.dram_tensor("out", [S, D_MODEL], sd, kind="ExternalOutput").ap()

    _dbg = bool(os.environ.get("KERNEL_DEBUG"))
    if _dbg:
        dbg_q0 = nc.dram_tensor("dbg_q0", [128, S], sd, kind="ExternalOutput").ap()
        dbg_q1 = nc.dram_tensor("dbg_q1", [128, S], sd, kind="ExternalOutput").ap()
        dbg_k = nc.dram_tensor("dbg_k", [128, S], sd, kind="ExternalOutput").ap()
        dbg_v = nc.dram_tensor("dbg_v", [128, KD, HEAD_DIM], sd, kind="ExternalOutput").ap()
        dbg_o0 = nc.dram_tensor("dbg_o0", [128, S], sd, kind="ExternalOutput").ap()
        dbg_o1 = nc.dram_tensor("dbg_o1", [128, S], sd, kind="ExternalOutput").ap()

    with tile.TileContext(nc) as tc:
        with tc.tile_pool(name="persist", bufs=1) as persist:
            # q0, q1, k transposed [128 hd, S]; start as pre-rope, finalized in place
            qkv = [persist.tile([128, S], sd, tag=f"qkv{m}", name=f"qkv{m}") for m in range(3)]
            vnat = persist.tile([128, KD, HEAD_DIM], sd, tag="vnat")
            oT = [persist.tile([128, S], sd, tag=f"oT{h}", name=f"oT{h}") for h in range(2)]
            ones_mat = persist.tile([128, 128], sd, tag="ones_mat")
            nc.vector.memset(ones_mat, 1.0)
            ident = persist.tile([128, 128], sd, tag="ident")
            make_identity(nc, ident)
            mean_col = persist.tile([128, 1], sd, tag="mean_col")
            nc.vector.memset(mean_col, 1.0 / HEAD_DIM)
            eps1 = persist.tile([1, 1], f32, tag="eps1")
            nc.vector.memset(eps1, EPS)
            eps128 = persist.tile([128, 1], f32, tag="eps128")
            nc.vector.memset(eps128, HEAD_DIM * EPS)
            # per-kv-block softmax scale: sclK[p, b] = SCALE / rms(k tok 128b+p)
            sclK = persist.tile([128, KD], f32, tag="sclK")

            ab_pools = (
                tc.tile_pool(name="rotp", bufs=1),
                tc.tile_pool(name="aw", bufs=1),
                tc.tile_pool(name="ax", bufs=1),
                tc.tile_pool(name="avt", bufs=2),
                tc.tile_pool(name="btmp", bufs=2),
                tc.tile_pool(name="bsm", bufs=2),
                tc.tile_pool(name="apsum", bufs=3, space="PSUM"),
                tc.tile_pool(name="bps", bufs=1, space="PSUM"),
                tc.tile_pool(name="bpsT", bufs=1, space="PSUM"),
            )
            rotp, awp, axp, avt, btp, bsm, apsum, bps, bpsT = (p.__enter__() for p in ab_pools)
            rot = [rotp.tile([128, S], sd, tag=f"rot{m}", name=f"rot{m}") for m in range(3)]
            wcat_sb = awp.tile([128, KD, 512], sd)
            xt_sb = axp.tile([128, KD, S], sd)
            # few large DMAs (issue on the sync queue costs ~0.6us each),
            # in k order so the first projection pass streams with arrival
            for j in range(4):
                nc.sync.dma_start(out=wcat_sb[:, 4 * j:4 * j + 4, :],
                                  in_=wcat_d[:, 4 * j:4 * j + 4, :])
                nc.sync.dma_start(out=xt_sb[:, 4 * j:4 * j + 2, :],
                                  in_=xt_d[:, 4 * j:4 * j + 2, :])
                nc.sync.dma_start(out=xt_sb[:, 4 * j + 2:4 * j + 4, :],
                                  in_=xt_d[:, 4 * j + 2:4 * j + 4, :])

            # warm the ACT function tables while projections run
            dummy = persist.tile([1, 1], f32, tag="dummy")
            nc.scalar.activation(dummy, eps1, AF.Square)
            nc.scalar.activation(dummy, eps1, AF.Sqrt)
            nc.scalar.activation(dummy, eps1, AF.Exp)

            # constants for phases B/C/D (after the critical-path loads)
            masks_sb = persist.tile([128, 8, 512], sd, tag="masks_sb")
            nc.sync.dma_start(out=masks_sb, in_=masks_d)
            wot_sb = persist.tile([128, 2, D_MODEL], sd, tag="wot_sb")
            nc.sync.dma_start(out=wot_sb, in_=wot_d)
            cs_sb = {}
            for nm, dd in (("cs2q", cs2q_d), ("ss2q", ss2q_d),
                           ("cs2k", cs2k_d), ("ss2k", ss2k_d)):
                t_ = persist.tile([128, S], sd, tag=nm, name=nm)
                nc.sync.dma_start(out=t_, in_=dd)
                cs_sb[nm] = t_

            def proj_pass(m):
                """project one 128-row slice of W_cat: 64 matmuls + eviction"""
                for half in range(2):
                    Y = apsum.tile([128, 1024], f32, tag="Y", name="Y")
                    for k in range(KD):
                        for n2 in range(2):
                            n = 2 * half + n2
                            nc.tensor.matmul(
                                Y[:, n2 * 512:(n2 + 1) * 512],
                                lhsT=wcat_sb[:, k, m * 128:(m + 1) * 128],
                                rhs=xt_sb[:, k, n * 512:(n + 1) * 512],
                                start=(k == 0),
                                stop=(k == KD - 1),
                            )
                    hs = slice(half * 1024, (half + 1) * 1024)
                    if m != 3:
                        if half == 0:
                            nc.scalar.copy(qkv[m][:, hs], Y)
                        else:
                            nc.vector.tensor_copy(qkv[m][:, hs], Y)
                    else:
                        # v: transpose 128x128 blocks into natural [S, hd]
                        vtmp = avt.tile([128, 1024], sd, tag="vtmp")
                        nc.scalar.copy(vtmp, Y)
                        tp = apsum.tile([128, 1024], sd, tag="Y", name="tpY")
                        for j in range(8):
                            nc.tensor.transpose(
                                tp[:, j * 128:(j + 1) * 128],
                                vtmp[:, j * 128:(j + 1) * 128], ident,
                            )
                        nc.vector.tensor_copy(vnat[:, half * 8:(half + 1) * 8, :], tp)
                if m != 3:
                    nc.sync.dma_start(out=rot[m][64:128, :], in_=qkv[m][0:64, :])
                    nc.sync.dma_start(out=rot[m][0:64, :], in_=qkv[m][64:128, :])

            def norm_rope(m):
                """rms norm + rope, in place on qkv[m].

                q (m<2): rn is needed per qT COLUMN -> row chain: [1,S] rms,
                reciprocal in a folded [128,16] view (InstReciprocal cost
                scales with FREE size: ~13us at 512+, 0.2us at 16), unfold,
                partition_broadcast, multiply into qT.

                k (m=2): rn lands per PARTITION in the score layout, so it is
                deferred to the exp: sclK[p,b] = 1/sqrt(sum_sq + 128*eps)
                = SCALE/rms, computed via 16 transposed n=1 matmuls. Kills
                k's DMA/broadcast chain entirely (rope commutes with the
                per-token scalar rn).
                """
                pre = qkv[m]
                rt = rot[m]
                csx = cs_sb["cs2q" if m < 2 else "cs2k"]
                ssx = cs_sb["ss2q" if m < 2 else "ss2k"]
                if m == 2:
                    psT = bpsT.tile([128, KD], f32, tag="psT")
                    for t in range(NQT):
                        ts_ = slice(t * 512, (t + 1) * 512)
                        sq = btp.tile([128, 512], sd, tag="sq")
                        nc.scalar.activation(sq, pre[:, ts_], AF.Square)
                        for j in range(4):
                            b = 4 * t + j
                            nc.tensor.matmul(
                                psT[:, b:b + 1],
                                lhsT=sq[:, j * 128:(j + 1) * 128],
                                rhs=ones_mat[:, 0:1],
                            )
                    nc.scalar.activation(sclK, psT, AF.Sqrt, bias=eps128)
                    nc.vector.reciprocal(sclK, sclK)
                else:
                    ss_sb = bsm.tile([1, S], f32, tag="ss")
                    for t in range(NQT):
                        ts_ = slice(t * 512, (t + 1) * 512)
                        sq = btp.tile([128, 512], sd, tag="sq")
                        nc.scalar.activation(sq, pre[:, ts_], AF.Square)
                        ps = bps.tile([1, 512], f32, tag="ssps")
                        nc.tensor.matmul(ps, lhsT=mean_col, rhs=sq)
                        nc.scalar.activation(ss_sb[0:1, ts_], ps, AF.Sqrt, bias=eps1)
                    ssT = bsm.tile([128, KD], f32, tag="ssT")
                    nc.sync.dma_start(out=ssT, in_=ss_sb)
                    nc.vector.reciprocal(ssT, ssT)
                    nc.sync.dma_start(out=ss_sb, in_=ssT)
                    rnb = btp.tile([128, S], f32, tag="rnb")
                    nc.gpsimd.partition_broadcast(rnb, ss_sb)
                # rope: final = (pre*cs + rot*ss) [* rn for q]
                t1 = btp.tile([128, S], sd, tag="t1")
                nc.vector.tensor_mul(t1, pre, csx)
                t2 = btp.tile([128, S], sd, tag="t2")
                nc.vector.tensor_mul(t2, rt, ssx)
                nc.vector.tensor_add(pre, t1, t2)
                if m != 2:
                    nc.vector.tensor_mul(pre, pre, rnb)

            def proj_pass_k_outer(m):
                """first projection pass: k outer so PE consumption paces the
                incoming xt DMA stream instead of sweeping ahead of it"""
                Ys = [apsum.tile([128, 1024], f32, tag="Y", name=f"Yk{h}")
                      for h in range(2)]
                for k in range(KD):
                    for half in range(2):
                        for n2 in range(2):
                            n = 2 * half + n2
                            nc.tensor.matmul(
                                Ys[half][:, n2 * 512:(n2 + 1) * 512],
                                lhsT=wcat_sb[:, k, m * 128:(m + 1) * 128],
                                rhs=xt_sb[:, k, n * 512:(n + 1) * 512],
                                start=(k == 0),
                                stop=(k == KD - 1),
                            )
                nc.scalar.copy(qkv[m][:, 0:1024], Ys[0])
                nc.vector.tensor_copy(qkv[m][:, 1024:], Ys[1])
                nc.sync.dma_start(out=rot[m][64:128, :], in_=qkv[m][0:64, :])
                nc.sync.dma_start(out=rot[m][0:64, :], in_=qkv[m][64:128, :])

            # interleaved emission: each tensor completes while the next
            # projection keeps the tensor engine busy
            proj_pass_k_outer(2)      # k
            proj_pass(3)      # v (transposes overlap k's norm chain)
            norm_rope(2)
            proj_pass(0)      # q0
            norm_rope(0)
            proj_pass(1)      # q1
            norm_rope(1)

            for p in reversed(ab_pools):
                p.__exit__(None, None, None)

            if _dbg:
                nc.sync.dma_start(out=dbg_q0, in_=qkv[0])
                nc.sync.dma_start(out=dbg_q1, in_=qkv[1])
                nc.sync.dma_start(out=dbg_k, in_=qkv[2])
                nc.sync.dma_start(out=dbg_v, in_=vnat)

            # -------- Phase C: windowed attention + fused out projection -----
            kT = qkv[2]
            with (
                tc.tile_pool(name="cpt", bufs=3) as cptp,
                tc.tile_pool(name="csm", bufs=2) as csmp,
                tc.tile_pool(name="dout", bufs=2) as doutp,
                tc.tile_pool(name="csc", bufs=2, space="PSUM") as cscp,
                tc.tile_pool(name="cacc", bufs=2, space="PSUM") as caccp,
                tc.tile_pool(name="cden", bufs=2, space="PSUM") as cdenp,
                tc.tile_pool(name="dps", bufs=2, space="PSUM") as dpsp,
            ):
                # software pipeline state: den/PV of block i are emitted after
                # QK of block i+1 (even across tile boundaries) so the
                # in-order PE queue never waits for the exp of block i.
                pending = []      # closures emitting den/PV of the previous block
                done_group = []   # (h, t, psO, psDb) awaiting normalize

                def flush():
                    for fn in pending:
                        fn()
                    pending.clear()

                def finalize():
                    for h, t, psO, psDb in done_group:
                        ts_ = slice(t * 512, (t + 1) * 512)
                        rbs = csmp.tile([128, 512], f32, tag="rbs")
                        # den is >= exp(diag) > 0 and O(1..1024): safe for the
                        # fast approx (18 bits, ~5x cheaper than reciprocal)
                        nc.vector.reciprocal_approx_fast(rbs, psDb)
                        nc.vector.tensor_mul(oT[h][:, ts_], psO, rbs)
                    done_group.clear()

                def attn_tile(h, t):
                    qT = qkv[h]
                    ts0 = t * 512
                    blocks = _blocks(t)
                    psO = caccp.tile([128, 512], f32, tag="psO")
                    psDb = cdenp.tile([128, 512], f32, tag="psDb")
                    for i, (b, off, wid, mi) in enumerate(blocks):
                        ps = cscp.tile([128, 512], f32, tag="sc")
                        nc.tensor.matmul(
                            ps[:, :wid],
                            lhsT=kT[:, b * 128:(b + 1) * 128],
                            rhs=qT[:, ts0 + off:ts0 + off + wid],
                        )
                        flush()
                        if i == 0:
                            finalize()
                        pt = cptp.tile([128, 512], sd, tag="pt")
                        nc.scalar.activation(pt[:, :wid], ps[:, :wid], AF.Exp,
                                             scale=sclK[:, b:b + 1])
                        if mi is not None:
                            nc.vector.tensor_mul(
                                pt[:, :wid], pt[:, :wid],
                                masks_sb[:, mi, off:off + wid],
                            )

                        first = (i == 0)
                        last = (i == len(blocks) - 1)

                        def emit_pv(b=b, off=off, wid=wid, pt=pt,
                                    psO=psO, psDb=psDb, first=first, last=last):
                            nc.tensor.matmul(
                                psDb[:, off:off + wid], lhsT=ones_mat,
                                rhs=pt[:, :wid],
                                start=first, stop=last, skip_group_check=True,
                            )
                            nc.tensor.matmul(
                                psO[:, off:off + wid], lhsT=vnat[:, b, :],
                                rhs=pt[:, :wid],
                                start=first, stop=last, skip_group_check=True,
                            )
                        pending.append(emit_pv)
                    done_group.append((h, t, psO, psDb))

                def outproj(t):
                    for tq in range(4 * t, 4 * t + 4):
                        ob = doutp.tile([128, D_MODEL], sd, tag="ob")
                        for dn in range(4):
                            po = dpsp.tile([128, 512], f32, tag="po")
                            nc.tensor.matmul(
                                po,
                                lhsT=oT[0][:, tq * 128:(tq + 1) * 128],
                                rhs=wot_sb[:, 0, dn * 512:(dn + 1) * 512],
                                start=True, stop=False,
                            )
                            nc.tensor.matmul(
                                po,
                                lhsT=oT[1][:, tq * 128:(tq + 1) * 128],
                                rhs=wot_sb[:, 1, dn * 512:(dn + 1) * 512],
                                start=False, stop=True,
                            )
                            ds_ = slice(dn * 512, (dn + 1) * 512)
                            # gpsimd cannot read PSUM; split across DVE/ACT
                            if dn % 2 == 0:
                                nc.vector.tensor_copy(ob[:, ds_], po)
                            else:
                                nc.scalar.copy(ob[:, ds_], po)
                        nc.sync.dma_start(out=out_d[tq * 128:(tq + 1) * 128, :], in_=ob)

                # attention tiles interleaved with out rows of finished tiles;
                # h0 tiles lead so q1's norm chain hides under them.
                # outproj(t) may only be emitted after a LATER attn_tile has
                # run finalize() for both (0,t) and (1,t) — emission order is
                # dependency order.
                attn_tile(0, 3)
                attn_tile(0, 0)
                attn_tile(1, 3)
                attn_tile(1, 0)   # finalize(1,3) runs at this tile's head
                outproj(3)
                attn_tile(0, 1)   # finalize(1,0)
                outproj(0)
                attn_tile(1, 1)
                attn_tile(0, 2)   # finalize(1,1)
                outproj(1)
                attn_tile(1, 2)
                flush()
                finalize()
                outproj(2)

                if _dbg:
                    nc.sync.dma_start(out=dbg_o0, in_=oT[0])
                    nc.sync.dma_start(out=dbg_o1, in_=oT[1])

    nc.compile()
    return nc


def _host_shards(x, wq, wk, wv, wo, q_norm_w, k_norm_w):
    if DTYPE_MODE == "bf16":
        import ml_dtypes
        sdt = ml_dtypes.bfloat16
    else:
        sdt = np.float32

    x2 = np.asarray(x, np.float32).reshape(S, D_MODEL)
    # [128, KD, S]: xt[p, k, s] = x[s, 128k+p]
    xt = np.ascontiguousarray(
        x2.T.reshape(KD, 128, S).transpose(1, 0, 2)
    ).astype(sdt)

    inv = 1.0 / (THETA ** (np.arange(0, HEAD_DIM, 2, dtype=np.float64) / HEAD_DIM))
    ang = np.arange(S, dtype=np.float64)[:, None] * inv[None, :]  # [S, 64]
    cos = np.cos(ang).T.astype(np.float32)  # [64, S]
    sin = np.sin(ang).T.astype(np.float32)

    def cs_ss(w):
        w = np.asarray(w, np.float32)
        wrot = np.concatenate([w[64:], w[:64]])
        cs2 = np.concatenate([cos, cos], 0) * w[:, None]
        ss2 = np.concatenate([-sin, sin], 0) * wrot[:, None]
        return np.ascontiguousarray(cs2).astype(sdt), np.ascontiguousarray(ss2).astype(sdt)

    cs2q, ss2q = cs_ss(q_norm_w)
    cs2k, ss2k = cs_ss(k_norm_w)

    masks = np.zeros((8, 128, 512), np.float32)
    ii = np.arange(128)[:, None]
    jj = np.arange(512)[None, :]
    for mi, d0 in enumerate(_MASK_D0):
        d = d0 + jj - ii
        masks[mi] = ((d >= 0) & (d < WINDOW)).astype(np.float32)
    masks_t = np.ascontiguousarray(masks.transpose(1, 0, 2)).astype(sdt)  # [128, 8, 512]

    wq = np.asarray(wq, np.float32)
    wk = np.asarray(wk, np.float32)
    wv = np.asarray(wv, np.float32)
    wo = np.asarray(wo, np.float32)

    in_maps = []
    for c in range(N_CORES):
        g = c // 2
        wcat = np.concatenate(
            [wq[256 * c:256 * (c + 1)], wk[128 * g:128 * (g + 1)], wv[128 * g:128 * (g + 1)]],
            axis=0,
        )  # [512, D]
        wcat_t = np.ascontiguousarray(
            wcat.T.reshape(KD, 128, 512).transpose(1, 0, 2)
        ).astype(sdt)  # [128, KD, 512]
        wot_t = np.ascontiguousarray(
            wo[:, 256 * c:256 * (c + 1)].T.reshape(2, 128, D_MODEL).transpose(1, 0, 2)
        ).astype(sdt)  # [128, 2, D]
        in_maps.append({
            "xt": xt,
            "wcat": wcat_t,
            "wot": wot_t,
            "cs2q": cs2q, "ss2q": ss2q, "cs2k": cs2k, "ss2k": ss2k,
            "masks": masks_t,
        })
    return in_maps


_NC_CACHE = None


def run_with_results(x, wq, wk, wv, wo, q_norm_w, k_norm_w, trace=False):
    global _NC_CACHE
    from concourse.bass_utils import run_bass_kernel_spmd

    if _NC_CACHE is None:
        _NC_CACHE = _build_program()
    nc = _NC_CACHE
    in_maps = _host_shards(x, wq, wk, wv, wo, q_norm_w, k_norm_w)
    res = run_bass_kernel_spmd(nc, in_maps, list(range(N_CORES)), trace=trace)
    parts = np.stack([np.asarray(res.results[i]["out"], np.float32) for i in range(N_CORES)], axis=0)
    out = parts.sum(axis=0, dtype=np.float32).reshape(B, S, D_MODEL)
    return np.ascontiguousarray(out.astype(np.float32)), res


def kernel(x, wq, wk, wv, wo, q_norm_w, k_norm_w):
    out, _ = run_with_results(x, wq, wk, wv, wo, q_norm_w, k_norm_w, trace=False)
    return out


# revision 27
# speedup vs baseline: 1.0470x; 1.0360x over previous
"""Trainium2 Bass kernel for sliding-window GQA attention block.

Module: q/k/v projections -> per-head RMSNorm(q,k) -> RoPE -> sliding-window
causal attention (window=1024, GQA 16 q heads / 4 kv heads) -> out projection.

Sharding (8 cores, tensor parallel over heads):
  core c owns q heads {2c, 2c+1} and kv head c//2.
  Each core computes attention for its 2 heads and a partial out-projection
  (contraction over its 256 head-features); the host sums the 8 partials.

Layout strategy on-chip (per core):
  - x is passed transposed (xt [D, S]) so QKV projection produces
    qT/kT [head_dim=128 partitions, S free] directly with wide N=512 matmuls.
  - v is produced transposed too, then PE-transposed into natural [S, 128]
    blocks (needed as the stationary operand of the PV matmul).
  - scores are computed transposed: sT [kv, q] = K_blk @ Q^T so that
    P^T (post-exp) is directly the moving operand for PV.  The softmax
    denominator uses an all-ones [128,128] stationary operand, which both
    sums over kv AND broadcasts the result across all 128 partitions, so
    normalization is a purely local reciprocal+multiply (no DRAM bounce).
  - RMS norm + RoPE run in the transposed layout: rotate-half is applied via
    a partition-rotated copy (DMA) plus host-precomputed cos/sin tables that
    also fold in the q/k norm weights; the 1/rms row is broadcast across
    partitions with gpsimd.partition_broadcast.
  - sliding-window structure: boundary / window-edge kv blocks only compute
    their valid q column range (column pruning), saving ~25% of attention
    matmul + exp work.
  - the partial out-projection is interleaved per-q-tile into the attention
    phase (attention h0/h1 of tile t, then out rows of tile t-1), and the
    attention inner loop is software-pipelined (den/PV of block i emitted
    after QK of block i+1) so the in-order PE queue never idles on the exp.

Precision: matmul-fed tensors are bf16 (set KERNEL_DTYPE=f32 for full fp32);
RMS/softmax-denominator chains are fp32; the partial output is written bf16.
"""

import os
import sys

for _p in ("/opt/trn_rl_repo", "/root/.axon_site/_ro/trn_rl_repo"):
    if _p not in sys.path:
        sys.path.insert(0, _p)

import numpy as np

N_HEADS = 16
N_KV_HEADS = 4
HEAD_DIM = 128
D_MODEL = 2048
WINDOW = 1024
THETA = 10000.0
EPS = 1e-6
S = 2048
B = 1
N_CORES = 8
KD = D_MODEL // 128          # 16 contraction tiles over d_model
NQT = S // 512               # 4 q tiles of 512
SCALE = HEAD_DIM ** -0.5

DTYPE_MODE = os.environ.get("KERNEL_DTYPE", "bf16")

# mask tile ids by delta0 = qstart - kvstart
_MASK_D0 = [0, -128, -256, -384, 640, 768, 896, 1024]
_MASK_IDX = {d0: i for i, d0 in enumerate(_MASK_D0)}


def _blocks(t):
    """kv blocks for q tile t: (b, q_off, q_wid, mask_idx), widest first.

    Column pruning: a boundary block (d0<=0) only reaches q >= -d0; a
    window-edge block (d0>=640) only reaches q < 1152-d0.  The first
    (full-width) block carries start=True and zeroes the whole psum range.
    """
    out = []
    for b in range(max(0, 4 * t - 8), 4 * t + 4):
        d0 = 512 * t - 128 * b
        if d0 <= 0:
            off, wid = -d0, 512 + d0
        elif d0 >= 640:
            off, wid = 0, 1152 - d0
        else:
            off, wid = 0, 512
        mi = _MASK_IDX.get(d0) if (d0 <= 0 or d0 > 512) else None
        out.append((b, off, wid, mi))
    out.sort(key=lambda x: -x[2])
    return out


def _build_program():
    import concourse.bass as bass
    import concourse.bacc as bacc
    import concourse.tile as tile
    from concourse import mybir
    from concourse.masks import make_identity

    f32 = mybir.dt.float32
    sd = mybir.dt.bfloat16 if DTYPE_MODE == "bf16" else f32
    AF = mybir.ActivationFunctionType

    nc = bacc.Bacc("TRN2", target_bir_lowering=False, debug=False)

    # host-pretiled: xt_t[p][k][s] = x[s, 128k+p] (partition-major so SBUF
    # loads are large contiguous 2D DMAs); wcat_t[p] holds the [kt, m]
    # weight tiles for partition p; similarly wot_t / masks.
    xt_d = nc.dram_tensor("xt", [128, KD, S], sd, kind="ExternalInput").ap()
    wcat_d = nc.dram_tensor("wcat", [128, KD, 512], sd, kind="ExternalInput").ap()
    wot_d = nc.dram_tensor("wot", [128, 2, D_MODEL], sd, kind="ExternalInput").ap()
    cs2q_d = nc.dram_tensor("cs2q", [128, S], sd, kind="ExternalInput").ap()
    ss2q_d = nc.dram_tensor("ss2q", [128, S], sd, kind="ExternalInput").ap()
    cs2k_d = nc.dram_tensor("cs2k", [128, S], sd, kind="ExternalInput").ap()
    ss2k_d = nc.dram_tensor("ss2k", [128, S], sd, kind="ExternalInput").ap()
    masks_d = nc.dram_tensor("masks", [128, 8, 512], sd, kind="ExternalInput").ap()
    out_d = nc.dram_tensor("out", [S, D_MODEL], sd, kind="ExternalOutput").ap()

    _dbg = bool(os.environ.get("KERNEL_DEBUG"))
    if _dbg:
        dbg_q0 = nc.dram_tensor("dbg_q0", [128, S], sd, kind="ExternalOutput").ap()
        dbg_q1 = nc.dram_tensor("dbg_q1", [128, S], sd, kind="ExternalOutput").ap()
        dbg_k = nc.dram_tensor("dbg_k", [128, S], sd, kind="ExternalOutput").ap()
        dbg_v = nc.dram_tensor("dbg_v", [128, KD, HEAD_DIM], sd, kind="ExternalOutput").ap()
        dbg_o0 = nc.dram_tensor("dbg_o0", [128, S], sd, kind="ExternalOutput").ap()
        dbg_o1 = nc.dram_tensor("dbg_o1", [128, S], sd, kind="ExternalOutput").ap()

    with tile.TileContext(nc) as tc:
        with tc.tile_pool(name="persist", bufs=1) as persist:
            # q0, q1, k transposed [128 hd, S]; start as pre-rope, finalized in place
            qkv = [persist.tile([128, S], sd, tag=f"qkv{m}", name=f"qkv{m}") for m in range(3)]
            vnat = persist.tile([128, KD, HEAD_DIM], sd, tag="vnat")
            oT = [persist.tile([128, S], sd, tag=f"oT{h}", name=f"oT{h}") for h in range(2)]
            ones_mat = persist.tile([128, 128], sd, tag="ones_mat")
            nc.vector.memset(ones_mat, 1.0)
            ident = persist.tile([128, 128], sd, tag="ident")
            make_identity(nc, ident)
            mean_col = persist.tile([128, 1], sd, tag="mean_col")
            nc.vector.memset(mean_col, 1.0 / HEAD_DIM)
            eps1 = persist.tile([1, 1], f32, tag="eps1")
            nc.vector.memset(eps1, EPS)
            eps128 = persist.tile([128, 1], f32, tag="eps128")
            nc.vector.memset(eps128, HEAD_DIM * EPS)
            # per-kv-block softmax scale: sclK[p, b] = SCALE / rms(k tok 128b+p)
            sclK = persist.tile([128, KD], f32, tag="sclK")

            ab_pools = (
                tc.tile_pool(name="rotp", bufs=1),
                tc.tile_pool(name="aw", bufs=1),
                tc.tile_pool(name="ax", bufs=1),
                tc.tile_pool(name="avt", bufs=2),
                tc.tile_pool(name="btmp", bufs=2),
                tc.tile_pool(name="bsm", bufs=1),
                tc.tile_pool(name="apsum", bufs=3, space="PSUM"),
                tc.tile_pool(name="bps", bufs=1, space="PSUM"),
                tc.tile_pool(name="bpsT", bufs=1, space="PSUM"),
            )
            rotp, awp, axp, avt, btp, bsm, apsum, bps, bpsT = (p.__enter__() for p in ab_pools)
            rot = [rotp.tile([128, S], sd, tag=f"rot{m}", name=f"rot{m}") for m in range(3)]
            wcat_sb = awp.tile([128, KD, 512], sd)
            xt_sb = axp.tile([128, KD, S], sd)
            # batched DMAs, small first (per-queue bw ~100GB/s, so early
            # chunks must be small and parallel; issues cost ~0.6us each)
            for wk, xk in ((slice(0, 1), slice(0, 1)),
                           (slice(1, 4), slice(1, 2)),
                           (None, slice(2, 4)),
                           (slice(4, 16), slice(4, 8)),
                           (None, slice(8, 12)),
                           (None, slice(12, 16))):
                if wk is not None:
                    nc.sync.dma_start(out=wcat_sb[:, wk, :], in_=wcat_d[:, wk, :])
                nc.sync.dma_start(out=xt_sb[:, xk, :], in_=xt_d[:, xk, :])

            # warm the ACT function tables while projections run
            dummy = persist.tile([1, 1], f32, tag="dummy")
            nc.scalar.activation(dummy, eps1, AF.Square)
            nc.scalar.activation(dummy, eps1, AF.Sqrt)
            nc.scalar.activation(dummy, eps1, AF.Exp)

            # constants for phases B/C/D (after the critical-path loads)
            masks_sb = persist.tile([128, 8, 512], sd, tag="masks_sb")
            nc.sync.dma_start(out=masks_sb, in_=masks_d)
            wot_sb = persist.tile([128, 2, D_MODEL], sd, tag="wot_sb")
            nc.sync.dma_start(out=wot_sb, in_=wot_d)
            cs_sb = {}
            for nm, dd in (("cs2q", cs2q_d), ("ss2q", ss2q_d),
                           ("cs2k", cs2k_d), ("ss2k", ss2k_d)):
                t_ = persist.tile([128, S], sd, tag=nm, name=nm)
                nc.sync.dma_start(out=t_, in_=dd)
                cs_sb[nm] = t_

            sq_of = {}

            def proj_pass(m):
                """project one 128-row slice of W_cat: 64 matmuls + eviction.

                For q/k tensors the squared values for the RMS norm are
                computed straight from PSUM (ACT Square) per half, so the
                norm's reduction matmuls are ready right at pass end instead
                of waiting on the SBUF eviction."""
                if m != 3:
                    sqm = btp.tile([128, S], sd, tag="sq", name=f"sq{m}")
                    sq_of[m] = sqm
                for half in range(2):
                    Y = apsum.tile([128, 1024], f32, tag="Y", name="Y")
                    for k in range(KD):
                        for n2 in range(2):
                            n = 2 * half + n2
                            nc.tensor.matmul(
                                Y[:, n2 * 512:(n2 + 1) * 512],
                                lhsT=wcat_sb[:, k, m * 128:(m + 1) * 128],
                                rhs=xt_sb[:, k, n * 512:(n + 1) * 512],
                                start=(k == 0),
                                stop=(k == KD - 1),
                            )
                    hs = slice(half * 1024, (half + 1) * 1024)
                    if m != 3:
                        if half == 0:
                            nc.scalar.copy(qkv[m][:, hs], Y)
                        else:
                            nc.vector.tensor_copy(qkv[m][:, hs], Y)
                        nc.scalar.activation(sqm[:, hs], Y, AF.Square)
                    else:
                        # v: transpose 128x128 blocks into natural [S, hd]
                        vtmp = avt.tile([128, 1024], sd, tag="vtmp")
                        nc.scalar.copy(vtmp, Y)
                        tp = apsum.tile([128, 1024], sd, tag="Y", name="tpY")
                        for j in range(8):
                            nc.tensor.transpose(
                                tp[:, j * 128:(j + 1) * 128],
                                vtmp[:, j * 128:(j + 1) * 128], ident,
                            )
                        nc.vector.tensor_copy(vnat[:, half * 8:(half + 1) * 8, :], tp)
                if m != 3:
                    nc.sync.dma_start(out=rot[m][64:128, :], in_=qkv[m][0:64, :])
                    nc.sync.dma_start(out=rot[m][0:64, :], in_=qkv[m][64:128, :])

            def norm_rope(m):
                """rms norm + rope, in place on qkv[m].

                q (m<2): rn is needed per qT COLUMN -> row chain: [1,S] rms,
                reciprocal in a folded [128,16] view (InstReciprocal cost
                scales with FREE size: ~13us at 512+, 0.2us at 16), unfold,
                partition_broadcast, multiply into qT.

                k (m=2): rn lands per PARTITION in the score layout, so it is
                deferred to the exp: sclK[p,b] = 1/sqrt(sum_sq + 128*eps)
                = SCALE/rms, computed via 16 transposed n=1 matmuls. Kills
                k's DMA/broadcast chain entirely (rope commutes with the
                per-token scalar rn).
                """
                pre = qkv[m]
                rt = rot[m]
                csx = cs_sb["cs2q" if m < 2 else "cs2k"]
                ssx = cs_sb["ss2q" if m < 2 else "ss2k"]
                sqm = sq_of.pop(m)
                if m == 2:
                    psT = bpsT.tile([128, KD], f32, tag="psT")
                    for b in range(KD):
                        nc.tensor.matmul(
                            psT[:, b:b + 1],
                            lhsT=sqm[:, b * 128:(b + 1) * 128],
                            rhs=ones_mat[:, 0:1],
                        )
                    nc.scalar.activation(sclK, psT, AF.Sqrt, bias=eps128)
                    nc.vector.reciprocal(sclK, sclK)
                else:
                    ss_sb = bsm.tile([1, S], f32, tag="ss")
                    for t in range(NQT):
                        ts_ = slice(t * 512, (t + 1) * 512)
                        ps = bps.tile([1, 512], f32, tag="ssps")
                        nc.tensor.matmul(ps, lhsT=mean_col, rhs=sqm[:, ts_])
                        nc.scalar.activation(ss_sb[0:1, ts_], ps, AF.Sqrt, bias=eps1)
                    ssT = bsm.tile([128, KD], f32, tag="ssT")
                    nc.sync.dma_start(out=ssT, in_=ss_sb)
                    nc.vector.reciprocal(ssT, ssT)
                    nc.sync.dma_start(out=ss_sb, in_=ssT)
                    rnb = btp.tile([128, S], f32, tag="rnb")
                    nc.gpsimd.partition_broadcast(rnb, ss_sb)
                # rope: final = (pre*cs + rot*ss) [* rn for q]
                t1 = btp.tile([128, S], sd, tag="t1")
                nc.vector.tensor_mul(t1, pre, csx)
                t2 = btp.tile([128, S], sd, tag="t2")
                nc.vector.tensor_mul(t2, rt, ssx)
                nc.vector.tensor_add(pre, t1, t2)
                if m != 2:
                    nc.vector.tensor_mul(pre, pre, rnb)

            def proj_pass_k_outer(m):
                """first projection pass: k outer so PE consumption paces the
                incoming xt DMA stream instead of sweeping ahead of it"""
                sqm = btp.tile([128, S], sd, tag="sq", name=f"sq{m}")
                sq_of[m] = sqm
                Ys = [apsum.tile([128, 1024], f32, tag="Y", name=f"Yk{h}")
                      for h in range(2)]
                for k in range(KD):
                    for half in range(2):
                        for n2 in range(2):
                            n = 2 * half + n2
                            nc.tensor.matmul(
                                Ys[half][:, n2 * 512:(n2 + 1) * 512],
                                lhsT=wcat_sb[:, k, m * 128:(m + 1) * 128],
                                rhs=xt_sb[:, k, n * 512:(n + 1) * 512],
                                start=(k == 0),
                                stop=(k == KD - 1),
                            )
                nc.scalar.copy(qkv[m][:, 0:1024], Ys[0])
                nc.vector.tensor_copy(qkv[m][:, 1024:], Ys[1])
                nc.scalar.activation(sqm[:, 0:1024], Ys[0], AF.Square)
                nc.scalar.activation(sqm[:, 1024:], Ys[1], AF.Square)
                nc.sync.dma_start(out=rot[m][64:128, :], in_=qkv[m][0:64, :])
                nc.sync.dma_start(out=rot[m][0:64, :], in_=qkv[m][64:128, :])

            # interleaved emission: each tensor completes while the next
            # projection keeps the tensor engine busy
            proj_pass_k_outer(2)      # k
            proj_pass(3)      # v (transposes overlap k's norm chain)
            norm_rope(2)
            proj_pass(0)      # q0
            norm_rope(0)
            proj_pass(1)      # q1
            norm_rope(1)

            for p in reversed(ab_pools):
                p.__exit__(None, None, None)

            if _dbg:
                nc.sync.dma_start(out=dbg_q0, in_=qkv[0])
                nc.sync.dma_start(out=dbg_q1, in_=qkv[1])
                nc.sync.dma_start(out=dbg_k, in_=qkv[2])
                nc.sync.dma_start(out=dbg_v, in_=vnat)

            # -------- Phase C: windowed attention + fused out projection -----
            kT = qkv[2]
            with (
                tc.tile_pool(name="cpt", bufs=3) as cptp,
                tc.tile_pool(name="csm", bufs=2) as csmp,
                tc.tile_pool(name="dout", bufs=2) as doutp,
                tc.tile_pool(name="csc", bufs=2, space="PSUM") as cscp,
                tc.tile_pool(name="cacc", bufs=2, space="PSUM") as caccp,
                tc.tile_pool(name="cden", bufs=2, space="PSUM") as cdenp,
                tc.tile_pool(name="dps", bufs=2, space="PSUM") as dpsp,
            ):
                # software pipeline state: den/PV of block i are emitted after
                # QK of block i+1 (even across tile boundaries) so the
                # in-order PE queue never waits for the exp of block i.
                pending = []      # closures emitting den/PV of the previous block
                done_group = []   # (h, t, psO, psDb) awaiting normalize

                def flush():
                    for fn in pending:
                        fn()
                    pending.clear()

                def finalize():
                    for h, t, psO, psDb in done_group:
                        ts_ = slice(t * 512, (t + 1) * 512)
                        rbs = csmp.tile([128, 512], f32, tag="rbs")
                        # den is >= exp(diag) > 0 and O(1..1024): safe for the
                        # fast approx (18 bits, ~5x cheaper than reciprocal)
                        nc.vector.reciprocal_approx_fast(rbs, psDb)
                        nc.vector.tensor_mul(oT[h][:, ts_], psO, rbs)
                    done_group.clear()

                def attn_tile(h, t):
                    qT = qkv[h]
                    ts0 = t * 512
                    blocks = _blocks(t)
                    psO = caccp.tile([128, 512], f32, tag="psO")
                    psDb = cdenp.tile([128, 512], f32, tag="psDb")
                    for i, (b, off, wid, mi) in enumerate(blocks):
                        ps = cscp.tile([128, 512], f32, tag="sc")
                        nc.tensor.matmul(
                            ps[:, :wid],
                            lhsT=kT[:, b * 128:(b + 1) * 128],
                            rhs=qT[:, ts0 + off:ts0 + off + wid],
                        )
                        flush()
                        if i == 0:
                            finalize()
                        pt = cptp.tile([128, 512], sd, tag="pt")
                        nc.scalar.activation(pt[:, :wid], ps[:, :wid], AF.Exp,
                                             scale=sclK[:, b:b + 1])
                        if mi is not None:
                            nc.vector.tensor_mul(
                                pt[:, :wid], pt[:, :wid],
                                masks_sb[:, mi, off:off + wid],
                            )

                        first = (i == 0)
                        last = (i == len(blocks) - 1)

                        def emit_pv(b=b, off=off, wid=wid, pt=pt,
                                    psO=psO, psDb=psDb, first=first, last=last):
                            nc.tensor.matmul(
                                psDb[:, off:off + wid], lhsT=ones_mat,
                                rhs=pt[:, :wid],
                                start=first, stop=last, skip_group_check=True,
                            )
                            nc.tensor.matmul(
                                psO[:, off:off + wid], lhsT=vnat[:, b, :],
                                rhs=pt[:, :wid],
                                start=first, stop=last, skip_group_check=True,
                            )
                        pending.append(emit_pv)
                    done_group.append((h, t, psO, psDb))

                def outproj(t):
                    for tq in range(4 * t, 4 * t + 4):
                        ob = doutp.tile([128, D_MODEL], sd, tag="ob")
                        for dn in range(4):
                            po = dpsp.tile([128, 512], f32, tag="po")
                            nc.tensor.matmul(
                                po,
                                lhsT=oT[0][:, tq * 128:(tq + 1) * 128],
                                rhs=wot_sb[:, 0, dn * 512:(dn + 1) * 512],
                                start=True, stop=False,
                            )
                            nc.tensor.matmul(
                                po,
                                lhsT=oT[1][:, tq * 128:(tq + 1) * 128],
                                rhs=wot_sb[:, 1, dn * 512:(dn + 1) * 512],
                                start=False, stop=True,
                            )
                            ds_ = slice(dn * 512, (dn + 1) * 512)
                            # all on DVE: ACT is loaded with the exps, and
                            # gpsimd cannot read PSUM
                            nc.vector.tensor_copy(ob[:, ds_], po)
                            if dn == 1:
                                nc.sync.dma_start(
                                    out=out_d[tq * 128:(tq + 1) * 128, 0:1024],
                                    in_=ob[:, 0:1024])
                        nc.sync.dma_start(out=out_d[tq * 128:(tq + 1) * 128, 1024:],
                                          in_=ob[:, 1024:])

                # attention tiles interleaved with out rows of finished tiles;
                # h0 tiles lead so q1's norm chain hides under them.
                # outproj(t) may only be emitted after a LATER attn_tile has
                # run finalize() for both (0,t) and (1,t) — emission order is
                # dependency order.
                attn_tile(0, 3)
                attn_tile(0, 0)
                attn_tile(1, 3)
                attn_tile(1, 0)   # finalize(1,3) runs at this tile's head
                outproj(3)
                attn_tile(0, 1)   # finalize(1,0)
                outproj(0)
                attn_tile(1, 1)
                attn_tile(0, 2)   # finalize(1,1)
                outproj(1)
                attn_tile(1, 2)
                flush()
                finalize()
                outproj(2)

                if _dbg:
                    nc.sync.dma_start(out=dbg_o0, in_=oT[0])
                    nc.sync.dma_start(out=dbg_o1, in_=oT[1])

    nc.compile()
    return nc


def _host_shards(x, wq, wk, wv, wo, q_norm_w, k_norm_w):
    if DTYPE_MODE == "bf16":
        import ml_dtypes
        sdt = ml_dtypes.bfloat16
    else:
        sdt = np.float32

    x2 = np.asarray(x, np.float32).reshape(S, D_MODEL)
    # [128, KD, S]: xt[p, k, s] = x[s, 128k+p]
    xt = np.ascontiguousarray(
        x2.T.reshape(KD, 128, S).transpose(1, 0, 2)
    ).astype(sdt)

    inv = 1.0 / (THETA ** (np.arange(0, HEAD_DIM, 2, dtype=np.float64) / HEAD_DIM))
    ang = np.arange(S, dtype=np.float64)[:, None] * inv[None, :]  # [S, 64]
    cos = np.cos(ang).T.astype(np.float32)  # [64, S]
    sin = np.sin(ang).T.astype(np.float32)

    def cs_ss(w):
        w = np.asarray(w, np.float32)
        wrot = np.concatenate([w[64:], w[:64]])
        cs2 = np.concatenate([cos, cos], 0) * w[:, None]
        ss2 = np.concatenate([-sin, sin], 0) * wrot[:, None]
        return np.ascontiguousarray(cs2).astype(sdt), np.ascontiguousarray(ss2).astype(sdt)

    cs2q, ss2q = cs_ss(q_norm_w)
    cs2k, ss2k = cs_ss(k_norm_w)

    masks = np.zeros((8, 128, 512), np.float32)
    ii = np.arange(128)[:, None]
    jj = np.arange(512)[None, :]
    for mi, d0 in enumerate(_MASK_D0):
        d = d0 + jj - ii
        masks[mi] = ((d >= 0) & (d < WINDOW)).astype(np.float32)
    masks_t = np.ascontiguousarray(masks.transpose(1, 0, 2)).astype(sdt)  # [128, 8, 512]

    wq = np.asarray(wq, np.float32)
    wk = np.asarray(wk, np.float32)
    wv = np.asarray(wv, np.float32)
    wo = np.asarray(wo, np.float32)

    in_maps = []
    for c in range(N_CORES):
        g = c // 2
        wcat = np.concatenate(
            [wq[256 * c:256 * (c + 1)], wk[128 * g:128 * (g + 1)], wv[128 * g:128 * (g + 1)]],
            axis=0,
        )  # [512, D]
        wcat_t = np.ascontiguousarray(
            wcat.T.reshape(KD, 128, 512).transpose(1, 0, 2)
        ).astype(sdt)  # [128, KD, 512]
        wot_t = np.ascontiguousarray(
            wo[:, 256 * c:256 * (c + 1)].T.reshape(2, 128, D_MODEL).transpose(1, 0, 2)
        ).astype(sdt)  # [128, 2, D]
        in_maps.append({
            "xt": xt,
            "wcat": wcat_t,
            "wot": wot_t,
            "cs2q": cs2q, "ss2q": ss2q, "cs2k": cs2k, "ss2k": ss2k,
            "masks": masks_t,
        })
    return in_maps


_NC_CACHE = None


def run_with_results(x, wq, wk, wv, wo, q_norm_w, k_norm_w, trace=False):
    global _NC_CACHE
    from concourse.bass_utils import run_bass_kernel_spmd

    if _NC_CACHE is None:
        _NC_CACHE = _build_program()
    nc = _NC_CACHE
    in_maps = _host_shards(x, wq, wk, wv, wo, q_norm_w, k_norm_w)
    res = run_bass_kernel_spmd(nc, in_maps, list(range(N_CORES)), trace=trace)
    parts = np.stack([np.asarray(res.results[i]["out"], np.float32) for i in range(N_CORES)], axis=0)
    out = parts.sum(axis=0, dtype=np.float32).reshape(B, S, D_MODEL)
    return np.ascontiguousarray(out.astype(np.float32)), res


def kernel(x, wq, wk, wv, wo, q_norm_w, k_norm_w):
    out, _ = run_with_results(x, wq, wk, wv, wo, q_norm_w, k_norm_w, trace=False)
    return out


# revision 29
# speedup vs baseline: 1.1288x; 1.0781x over previous
"""Trainium2 Bass kernel for sliding-window GQA attention block.

Module: q/k/v projections -> per-head RMSNorm(q,k) -> RoPE -> sliding-window
causal attention (window=1024, GQA 16 q heads / 4 kv heads) -> out projection.

Sharding (8 cores, tensor parallel over heads):
  core c owns q heads {2c, 2c+1} and kv head c//2.
  Each core computes attention for its 2 heads and a partial out-projection
  (contraction over its 256 head-features); the host sums the 8 partials.

Layout strategy on-chip (per core):
  - x is passed transposed (xt [D, S]) so QKV projection produces
    qT/kT [head_dim=128 partitions, S free] directly with wide N=512 matmuls.
  - v is produced transposed too, then PE-transposed into natural [S, 128]
    blocks (needed as the stationary operand of the PV matmul).
  - scores are computed transposed: sT [kv, q] = K_blk @ Q^T so that
    P^T (post-exp) is directly the moving operand for PV.  The softmax
    denominator uses an all-ones [128,128] stationary operand, which both
    sums over kv AND broadcasts the result across all 128 partitions, so
    normalization is a purely local reciprocal+multiply (no DRAM bounce).
  - RMS norm + RoPE run in the transposed layout: rotate-half is applied via
    a partition-rotated copy (DMA) plus host-precomputed cos/sin tables that
    also fold in the q/k norm weights; the 1/rms row is broadcast across
    partitions with gpsimd.partition_broadcast.
  - sliding-window structure: boundary / window-edge kv blocks only compute
    their valid q column range (column pruning), saving ~25% of attention
    matmul + exp work.
  - the partial out-projection is interleaved per-q-tile into the attention
    phase (attention h0/h1 of tile t, then out rows of tile t-1), and the
    attention inner loop is software-pipelined (den/PV of block i emitted
    after QK of block i+1) so the in-order PE queue never idles on the exp.

Precision: matmul-fed tensors are bf16 (set KERNEL_DTYPE=f32 for full fp32);
RMS/softmax-denominator chains are fp32; the partial output is written bf16.
"""

import os
import sys

for _p in ("/opt/trn_rl_repo", "/root/.axon_site/_ro/trn_rl_repo"):
    if _p not in sys.path:
        sys.path.insert(0, _p)

import numpy as np

N_HEADS = 16
N_KV_HEADS = 4
HEAD_DIM = 128
D_MODEL = 2048
WINDOW = 1024
THETA = 10000.0
EPS = 1e-6
S = 2048
B = 1
N_CORES = 8
KD = D_MODEL // 128          # 16 contraction tiles over d_model
NQT = S // 512               # 4 q tiles of 512
SCALE = HEAD_DIM ** -0.5

DTYPE_MODE = os.environ.get("KERNEL_DTYPE", "bf16")

# mask tile ids by delta0 = qstart - kvstart
_MASK_D0 = [0, -128, -256, -384, 640, 768, 896, 1024]
_MASK_IDX = {d0: i for i, d0 in enumerate(_MASK_D0)}


def _blocks(t):
    """kv blocks for q tile t: (b, q_off, q_wid, mask_idx), widest first.

    Column pruning: a boundary block (d0<=0) only reaches q >= -d0; a
    window-edge block (d0>=640) only reaches q < 1152-d0.  The first
    (full-width) block carries start=True and zeroes the whole psum range.
    """
    out = []
    for b in range(max(0, 4 * t - 8), 4 * t + 4):
        d0 = 512 * t - 128 * b
        if d0 <= 0:
            off, wid = -d0, 512 + d0
        elif d0 >= 640:
            off, wid = 0, 1152 - d0
        else:
            off, wid = 0, 512
        mi = _MASK_IDX.get(d0) if (d0 <= 0 or d0 > 512) else None
        out.append((b, off, wid, mi))
    out.sort(key=lambda x: -x[2])
    return out


def _build_program():
    import concourse.bass as bass
    import concourse.bacc as bacc
    import concourse.tile as tile
    from concourse import mybir
    from concourse.masks import make_identity

    f32 = mybir.dt.float32
    sd = mybir.dt.bfloat16 if DTYPE_MODE == "bf16" else f32
    AF = mybir.ActivationFunctionType

    nc = bacc.Bacc("TRN2", target_bir_lowering=False, debug=False)

    # host-pretiled: xt_t[p][k][s] = x[s, 128k+p] (partition-major so SBUF
    # loads are large contiguous 2D DMAs); wcat_t[p] holds the [kt, m]
    # weight tiles for partition p; similarly wot_t / masks.
    xt_d = nc.dram_tensor("xt", [128, KD, S], sd, kind="ExternalInput").ap()
    wcat_d = nc.dram_tensor("wcat", [128, KD, 512], sd, kind="ExternalInput").ap()
    wot_d = nc.dram_tensor("wot", [128, 2, D_MODEL], sd, kind="ExternalInput").ap()
    cs2q_d = nc.dram_tensor("cs2q", [128, S], sd, kind="ExternalInput").ap()
    ss2q_d = nc.dram_tensor("ss2q", [128, S], sd, kind="ExternalInput").ap()
    cs2k_d = nc.dram_tensor("cs2k", [128, S], sd, kind="ExternalInput").ap()
    ss2k_d = nc.dram_tensor("ss2k", [128, S], sd, kind="ExternalInput").ap()
    masks_d = nc.dram_tensor("masks", [128, 8, 512], sd, kind="ExternalInput").ap()
    out_d = nc.dram_tensor("out", [S, D_MODEL], sd, kind="ExternalOutput").ap()

    _dbg = bool(os.environ.get("KERNEL_DEBUG"))
    if _dbg:
        dbg_q0 = nc.dram_tensor("dbg_q0", [128, S], sd, kind="ExternalOutput").ap()
        dbg_q1 = nc.dram_tensor("dbg_q1", [128, S], sd, kind="ExternalOutput").ap()
        dbg_k = nc.dram_tensor("dbg_k", [128, S], sd, kind="ExternalOutput").ap()
        dbg_v = nc.dram_tensor("dbg_v", [128, KD, HEAD_DIM], sd, kind="ExternalOutput").ap()
        dbg_o0 = nc.dram_tensor("dbg_o0", [128, S], sd, kind="ExternalOutput").ap()
        dbg_o1 = nc.dram_tensor("dbg_o1", [128, S], sd, kind="ExternalOutput").ap()

    with tile.TileContext(nc) as tc:
        with tc.tile_pool(name="persist", bufs=1) as persist:
            # q0, q1, k transposed [128 hd, S]; start as pre-rope, finalized in place
            qkv = [persist.tile([128, S], sd, tag=f"qkv{m}", name=f"qkv{m}") for m in range(3)]
            vnat = persist.tile([128, KD, HEAD_DIM], sd, tag="vnat")
            oT = [persist.tile([128, S], sd, tag=f"oT{h}", name=f"oT{h}") for h in range(2)]
            ones_mat = persist.tile([128, 128], sd, tag="ones_mat")
            nc.vector.memset(ones_mat, 1.0)
            ident = persist.tile([128, 128], sd, tag="ident")
            make_identity(nc, ident)
            mean_col = persist.tile([128, 1], sd, tag="mean_col")
            nc.vector.memset(mean_col, 1.0 / HEAD_DIM)
            eps1 = persist.tile([1, 1], f32, tag="eps1")
            nc.vector.memset(eps1, EPS)
            eps128 = persist.tile([128, 1], f32, tag="eps128")
            nc.vector.memset(eps128, HEAD_DIM * EPS)
            # per-kv-block softmax scale: sclK[p, b] = SCALE / rms(k tok 128b+p)
            sclK = persist.tile([128, KD], f32, tag="sclK")

            ab_pools = (
                tc.tile_pool(name="rotp", bufs=1),
                tc.tile_pool(name="aw", bufs=1),
                tc.tile_pool(name="ax", bufs=1),
                tc.tile_pool(name="avt", bufs=2),
                tc.tile_pool(name="btmp", bufs=2),
                tc.tile_pool(name="bsm", bufs=1),
                tc.tile_pool(name="apsum", bufs=3, space="PSUM"),
                tc.tile_pool(name="bps", bufs=1, space="PSUM"),
                tc.tile_pool(name="bpsT", bufs=1, space="PSUM"),
            )
            rotp, awp, axp, avt, btp, bsm, apsum, bps, bpsT = (p.__enter__() for p in ab_pools)
            rot = [rotp.tile([128, S], sd, tag=f"rot{m}", name=f"rot{m}") for m in range(3)]
            wcat_sb = awp.tile([128, KD, 512], sd)
            xt_sb = axp.tile([128, KD, S], sd)
            # per-k DMAs: each queue runs ~100-130GB/s, so many small
            # parallel transfers beat few big ones (a 2MB chunk on one
            # queue takes ~20us and starves the first projection pass)
            for k in range(KD):
                nc.sync.dma_start(out=wcat_sb[:, k, :], in_=wcat_d[:, k, :])
                nc.sync.dma_start(out=xt_sb[:, k, :], in_=xt_d[:, k, :])

            # warm the ACT function tables while projections run
            dummy = persist.tile([1, 1], f32, tag="dummy")
            nc.scalar.activation(dummy, eps1, AF.Square)
            nc.scalar.activation(dummy, eps1, AF.Sqrt)
            nc.scalar.activation(dummy, eps1, AF.Exp)

            # constants for phases B/C/D (after the critical-path loads)
            masks_sb = persist.tile([128, 8, 512], sd, tag="masks_sb")
            nc.sync.dma_start(out=masks_sb, in_=masks_d)
            wot_sb = persist.tile([128, 2, D_MODEL], sd, tag="wot_sb")
            nc.sync.dma_start(out=wot_sb, in_=wot_d)
            cs_sb = {}
            for nm, dd in (("cs2q", cs2q_d), ("ss2q", ss2q_d),
                           ("cs2k", cs2k_d), ("ss2k", ss2k_d)):
                t_ = persist.tile([128, S], sd, tag=nm, name=nm)
                nc.sync.dma_start(out=t_, in_=dd)
                cs_sb[nm] = t_

            sq_of = {}

            def proj_pass(m):
                """project one 128-row slice of W_cat: 64 matmuls + eviction.

                For q/k tensors the squared values for the RMS norm are
                computed straight from PSUM (ACT Square) per half, so the
                norm's reduction matmuls are ready right at pass end instead
                of waiting on the SBUF eviction."""
                if m != 3:
                    sqm = btp.tile([128, S], sd, tag="sq", name=f"sq{m}")
                    sq_of[m] = sqm
                for half in range(2):
                    Y = apsum.tile([128, 1024], f32, tag="Y", name="Y")
                    for k in range(KD):
                        for n2 in range(2):
                            n = 2 * half + n2
                            nc.tensor.matmul(
                                Y[:, n2 * 512:(n2 + 1) * 512],
                                lhsT=wcat_sb[:, k, m * 128:(m + 1) * 128],
                                rhs=xt_sb[:, k, n * 512:(n + 1) * 512],
                                start=(k == 0),
                                stop=(k == KD - 1),
                            )
                    hs = slice(half * 1024, (half + 1) * 1024)
                    if m != 3:
                        if half == 0:
                            nc.scalar.copy(qkv[m][:, hs], Y)
                        else:
                            nc.vector.tensor_copy(qkv[m][:, hs], Y)
                        nc.scalar.activation(sqm[:, hs], Y, AF.Square)
                    else:
                        # v: transpose 128x128 blocks into natural [S, hd]
                        vtmp = avt.tile([128, 1024], sd, tag="vtmp")
                        nc.scalar.copy(vtmp, Y)
                        tp = apsum.tile([128, 1024], sd, tag="Y", name="tpY")
                        for j in range(8):
                            nc.tensor.transpose(
                                tp[:, j * 128:(j + 1) * 128],
                                vtmp[:, j * 128:(j + 1) * 128], ident,
                            )
                        nc.vector.tensor_copy(vnat[:, half * 8:(half + 1) * 8, :], tp)
                if m != 3:
                    nc.sync.dma_start(out=rot[m][64:128, :], in_=qkv[m][0:64, :])
                    nc.sync.dma_start(out=rot[m][0:64, :], in_=qkv[m][64:128, :])

            def norm_rope(m):
                """rms norm + rope, in place on qkv[m].

                q (m<2): rn is needed per qT COLUMN -> row chain: [1,S] rms,
                reciprocal in a folded [128,16] view (InstReciprocal cost
                scales with FREE size: ~13us at 512+, 0.2us at 16), unfold,
                partition_broadcast, multiply into qT.

                k (m=2): rn lands per PARTITION in the score layout, so it is
                deferred to the exp: sclK[p,b] = 1/sqrt(sum_sq + 128*eps)
                = SCALE/rms, computed via 16 transposed n=1 matmuls. Kills
                k's DMA/broadcast chain entirely (rope commutes with the
                per-token scalar rn).
                """
                pre = qkv[m]
                rt = rot[m]
                csx = cs_sb["cs2q" if m < 2 else "cs2k"]
                ssx = cs_sb["ss2q" if m < 2 else "ss2k"]
                sqm = sq_of.pop(m)
                if m == 2:
                    psT = bpsT.tile([128, KD], f32, tag="psT")
                    for b in range(KD):
                        nc.tensor.matmul(
                            psT[:, b:b + 1],
                            lhsT=sqm[:, b * 128:(b + 1) * 128],
                            rhs=ones_mat[:, 0:1],
                        )
                    nc.scalar.activation(sclK, psT, AF.Sqrt, bias=eps128)
                    nc.vector.reciprocal(sclK, sclK)
                else:
                    ss_sb = bsm.tile([1, S], f32, tag="ss")
                    for t in range(NQT):
                        ts_ = slice(t * 512, (t + 1) * 512)
                        ps = bps.tile([1, 512], f32, tag="ssps")
                        nc.tensor.matmul(ps, lhsT=mean_col, rhs=sqm[:, ts_])
                        nc.scalar.activation(ss_sb[0:1, ts_], ps, AF.Sqrt, bias=eps1)
                    ssT = bsm.tile([128, KD], f32, tag="ssT")
                    nc.sync.dma_start(out=ssT, in_=ss_sb)
                    nc.vector.reciprocal(ssT, ssT)
                    nc.sync.dma_start(out=ss_sb, in_=ssT)
                    rnb = btp.tile([128, S], f32, tag="rnb")
                    nc.gpsimd.partition_broadcast(rnb, ss_sb)
                # rope: final = (pre*cs + rot*ss) [* rn for q]
                t1 = btp.tile([128, S], sd, tag="t1")
                nc.vector.tensor_mul(t1, pre, csx)
                t2 = btp.tile([128, S], sd, tag="t2")
                nc.vector.tensor_mul(t2, rt, ssx)
                nc.vector.tensor_add(pre, t1, t2)
                if m != 2:
                    nc.vector.tensor_mul(pre, pre, rnb)

            def proj_pass_k_outer(m):
                """first projection pass: k outer so PE consumption paces the
                incoming xt DMA stream instead of sweeping ahead of it"""
                sqm = btp.tile([128, S], sd, tag="sq", name=f"sq{m}")
                sq_of[m] = sqm
                Ys = [apsum.tile([128, 1024], f32, tag="Y", name=f"Yk{h}")
                      for h in range(2)]
                for k in range(KD):
                    for half in range(2):
                        for n2 in range(2):
                            n = 2 * half + n2
                            nc.tensor.matmul(
                                Ys[half][:, n2 * 512:(n2 + 1) * 512],
                                lhsT=wcat_sb[:, k, m * 128:(m + 1) * 128],
                                rhs=xt_sb[:, k, n * 512:(n + 1) * 512],
                                start=(k == 0),
                                stop=(k == KD - 1),
                            )
                nc.scalar.copy(qkv[m][:, 0:1024], Ys[0])
                nc.vector.tensor_copy(qkv[m][:, 1024:], Ys[1])
                nc.scalar.activation(sqm[:, 0:1024], Ys[0], AF.Square)
                nc.scalar.activation(sqm[:, 1024:], Ys[1], AF.Square)
                nc.sync.dma_start(out=rot[m][64:128, :], in_=qkv[m][0:64, :])
                nc.sync.dma_start(out=rot[m][0:64, :], in_=qkv[m][64:128, :])

            # interleaved emission: each tensor completes while the next
            # projection keeps the tensor engine busy
            proj_pass_k_outer(2)      # k
            proj_pass(3)      # v (transposes overlap k's norm chain)
            norm_rope(2)
            proj_pass(0)      # q0
            norm_rope(0)
            proj_pass(1)      # q1
            norm_rope(1)

            for p in reversed(ab_pools):
                p.__exit__(None, None, None)

            if _dbg:
                nc.sync.dma_start(out=dbg_q0, in_=qkv[0])
                nc.sync.dma_start(out=dbg_q1, in_=qkv[1])
                nc.sync.dma_start(out=dbg_k, in_=qkv[2])
                nc.sync.dma_start(out=dbg_v, in_=vnat)

            # -------- Phase C: windowed attention + fused out projection -----
            kT = qkv[2]
            with (
                tc.tile_pool(name="cpt", bufs=3) as cptp,
                tc.tile_pool(name="csm", bufs=2) as csmp,
                tc.tile_pool(name="dout", bufs=2) as doutp,
                tc.tile_pool(name="csc", bufs=2, space="PSUM") as cscp,
                tc.tile_pool(name="cacc", bufs=2, space="PSUM") as caccp,
                tc.tile_pool(name="cden", bufs=2, space="PSUM") as cdenp,
                tc.tile_pool(name="dps", bufs=2, space="PSUM") as dpsp,
            ):
                # software pipeline state: den/PV of block i are emitted after
                # QK of block i+1 (even across tile boundaries) so the
                # in-order PE queue never waits for the exp of block i.
                pending = []      # closures emitting den/PV of the previous block
                done_group = []   # (h, t, psO, psDb) awaiting normalize

                def flush():
                    for fn in pending:
                        fn()
                    pending.clear()

                def finalize():
                    for h, t, psO, psDb in done_group:
                        ts_ = slice(t * 512, (t + 1) * 512)
                        rbs = csmp.tile([128, 512], f32, tag="rbs")
                        # den is >= exp(diag) > 0 and O(1..1024): safe for the
                        # fast approx (18 bits, ~5x cheaper than reciprocal)
                        nc.vector.reciprocal_approx_fast(rbs, psDb)
                        nc.vector.tensor_mul(oT[h][:, ts_], psO, rbs)
                    done_group.clear()

                def attn_tile(h, t):
                    qT = qkv[h]
                    ts0 = t * 512
                    blocks = _blocks(t)
                    psO = caccp.tile([128, 512], f32, tag="psO")
                    psDb = cdenp.tile([128, 512], f32, tag="psDb")
                    for i, (b, off, wid, mi) in enumerate(blocks):
                        ps = cscp.tile([128, 512], f32, tag="sc")
                        nc.tensor.matmul(
                            ps[:, :wid],
                            lhsT=kT[:, b * 128:(b + 1) * 128],
                            rhs=qT[:, ts0 + off:ts0 + off + wid],
                        )
                        flush()
                        if i == 0:
                            finalize()
                        pt = cptp.tile([128, 512], sd, tag="pt")
                        nc.scalar.activation(pt[:, :wid], ps[:, :wid], AF.Exp,
                                             scale=sclK[:, b:b + 1])
                        if mi is not None:
                            nc.vector.tensor_mul(
                                pt[:, :wid], pt[:, :wid],
                                masks_sb[:, mi, off:off + wid],
                            )

                        first = (i == 0)
                        last = (i == len(blocks) - 1)

                        def emit_pv(b=b, off=off, wid=wid, pt=pt,
                                    psO=psO, psDb=psDb, first=first, last=last):
                            nc.tensor.matmul(
                                psDb[:, off:off + wid], lhsT=ones_mat,
                                rhs=pt[:, :wid],
                                start=first, stop=last, skip_group_check=True,
                            )
                            nc.tensor.matmul(
                                psO[:, off:off + wid], lhsT=vnat[:, b, :],
                                rhs=pt[:, :wid],
                                start=first, stop=last, skip_group_check=True,
                            )
                        pending.append(emit_pv)
                    done_group.append((h, t, psO, psDb))

                def outproj(t):
                    for tq in range(4 * t, 4 * t + 4):
                        ob = doutp.tile([128, D_MODEL], sd, tag="ob")
                        for dn in range(4):
                            po = dpsp.tile([128, 512], f32, tag="po")
                            nc.tensor.matmul(
                                po,
                                lhsT=oT[0][:, tq * 128:(tq + 1) * 128],
                                rhs=wot_sb[:, 0, dn * 512:(dn + 1) * 512],
                                start=True, stop=False,
                            )
                            nc.tensor.matmul(
                                po,
                                lhsT=oT[1][:, tq * 128:(tq + 1) * 128],
                                rhs=wot_sb[:, 1, dn * 512:(dn + 1) * 512],
                                start=False, stop=True,
                            )
                            ds_ = slice(dn * 512, (dn + 1) * 512)
                            # split DVE/ACT (gpsimd cannot read PSUM; all-DVE
                            # head-of-line-blocks the masks/finalize behind it)
                            if dn % 2 == 0:
                                nc.vector.tensor_copy(ob[:, ds_], po)
                            else:
                                nc.scalar.copy(ob[:, ds_], po)
                            if dn == 1:
                                nc.sync.dma_start(
                                    out=out_d[tq * 128:(tq + 1) * 128, 0:1024],
                                    in_=ob[:, 0:1024])
                        nc.sync.dma_start(out=out_d[tq * 128:(tq + 1) * 128, 1024:],
                                          in_=ob[:, 1024:])

                # attention tiles interleaved with out rows of finished tiles;
                # h0 tiles lead so q1's norm chain hides under them.
                # outproj(t) may only be emitted after a LATER attn_tile has
                # run finalize() for both (0,t) and (1,t) — emission order is
                # dependency order.
                attn_tile(0, 3)
                attn_tile(0, 0)
                attn_tile(1, 3)
                attn_tile(1, 0)   # finalize(1,3) runs at this tile's head
                outproj(3)
                attn_tile(0, 1)   # finalize(1,0)
                outproj(0)
                attn_tile(1, 1)
                attn_tile(0, 2)   # finalize(1,1)
                outproj(1)
                attn_tile(1, 2)
                flush()
                finalize()
                outproj(2)

                if _dbg:
                    nc.sync.dma_start(out=dbg_o0, in_=oT[0])
                    nc.sync.dma_start(out=dbg_o1, in_=oT[1])

    nc.compile()
    return nc


def _host_shards(x, wq, wk, wv, wo, q_norm_w, k_norm_w):
    if DTYPE_MODE == "bf16":
        import ml_dtypes
        sdt = ml_dtypes.bfloat16
    else:
        sdt = np.float32

    x2 = np.asarray(x, np.float32).reshape(S, D_MODEL)
    # [128, KD, S]: xt[p, k, s] = x[s, 128k+p]
    xt = np.ascontiguousarray(
        x2.T.reshape(KD, 128, S).transpose(1, 0, 2)
    ).astype(sdt)

    inv = 1.0 / (THETA ** (np.arange(0, HEAD_DIM, 2, dtype=np.float64) / HEAD_DIM))
    ang = np.arange(S, dtype=np.float64)[:, None] * inv[None, :]  # [S, 64]
    cos = np.cos(ang).T.astype(np.float32)  # [64, S]
    sin = np.sin(ang).T.astype(np.float32)

    def cs_ss(w):
        w = np.asarray(w, np.float32)
        wrot = np.concatenate([w[64:], w[:64]])
        cs2 = np.concatenate([cos, cos], 0) * w[:, None]
        ss2 = np.concatenate([-sin, sin], 0) * wrot[:, None]
        return np.ascontiguousarray(cs2).astype(sdt), np.ascontiguousarray(ss2).astype(sdt)

    cs2q, ss2q = cs_ss(q_norm_w)
    cs2k, ss2k = cs_ss(k_norm_w)

    masks = np.zeros((8, 128, 512), np.float32)
    ii = np.arange(128)[:, None]
    jj = np.arange(512)[None, :]
    for mi, d0 in enumerate(_MASK_D0):
        d = d0 + jj - ii
        masks[mi] = ((d >= 0) & (d < WINDOW)).astype(np.float32)
    masks_t = np.ascontiguousarray(masks.transpose(1, 0, 2)).astype(sdt)  # [128, 8, 512]

    wq = np.asarray(wq, np.float32)
    wk = np.asarray(wk, np.float32)
    wv = np.asarray(wv, np.float32)
    wo = np.asarray(wo, np.float32)

    in_maps = []
    for c in range(N_CORES):
        g = c // 2
        wcat = np.concatenate(
            [wq[256 * c:256 * (c + 1)], wk[128 * g:128 * (g + 1)], wv[128 * g:128 * (g + 1)]],
            axis=0,
        )  # [512, D]
        wcat_t = np.ascontiguousarray(
            wcat.T.reshape(KD, 128, 512).transpose(1, 0, 2)
        ).astype(sdt)  # [128, KD, 512]
        wot_t = np.ascontiguousarray(
            wo[:, 256 * c:256 * (c + 1)].T.reshape(2, 128, D_MODEL).transpose(1, 0, 2)
        ).astype(sdt)  # [128, 2, D]
        in_maps.append({
            "xt": xt,
            "wcat": wcat_t,
            "wot": wot_t,
            "cs2q": cs2q, "ss2q": ss2q, "cs2k": cs2k, "ss2k": ss2k,
            "masks": masks_t,
        })
    return in_maps


_NC_CACHE = None


def run_with_results(x, wq, wk, wv, wo, q_norm_w, k_norm_w, trace=False):
    global _NC_CACHE
    from concourse.bass_utils import run_bass_kernel_spmd

    if _NC_CACHE is None:
        _NC_CACHE = _build_program()
    nc = _NC_CACHE
    in_maps = _host_shards(x, wq, wk, wv, wo, q_norm_w, k_norm_w)
    res = run_bass_kernel_spmd(nc, in_maps, list(range(N_CORES)), trace=trace)
    parts = np.stack([np.asarray(res.results[i]["out"], np.float32) for i in range(N_CORES)], axis=0)
    out = parts.sum(axis=0, dtype=np.float32).reshape(B, S, D_MODEL)
    return np.ascontiguousarray(out.astype(np.float32)), res


def kernel(x, wq, wk, wv, wo, q_norm_w, k_norm_w):
    out, _ = run_with_results(x, wq, wk, wv, wo, q_norm_w, k_norm_w, trace=False)
    return out
